# revision 17
# baseline (speedup 1.0000x reference)
"""Trainium2 Bass kernel for nn_Actor_73057393705109.

Architecture (per stock s, sharded one stock per NeuronCore, 8 cores):
  TimeLSTM over T=30 steps of B*D=160 sequences (E=768 -> H=128)
  -> masked attention over T -> day-LSTM over D=5 -> attention over D
  -> 2-layer MLP head per stock -> AllGather -> global linear head.

Device layout: "transposed" everywhere — feature dims on SBUF partitions,
sequence index n = b*D + d on the free dim. Matmul operands in bf16
(1 cyc/row on the PE), state and softmax math in fp32.
"""

import sys

if "/opt/trn_rl_repo" not in sys.path:
    sys.path.insert(0, "/opt/trn_rl_repo")

import ml_dtypes
import numpy as np

import concourse.bacc as bacc
import concourse.bass as bass
import concourse.mybir as mybir
from concourse import library_config
from concourse.tile import TileContext

F32 = mybir.dt.float32
BF16 = mybir.dt.bfloat16
AF = mybir.ActivationFunctionType
OP = mybir.AluOpType
BF = ml_dtypes.bfloat16

S, B, D, T, E, H = 8, 32, 5, 30, 768, 128
H4 = 4 * H
N = B * D            # 160 sequences per stock
TOK = T * N          # 4800 tokens, t-major: tok = t*N + n
EB = E // 128        # 6 e-blocks
TPC = 3              # t-steps per xU chunk
CH = TPC * N         # 480 tokens per chunk
NCH = T // TPC       # 10 chunks
NCORES = 8
import os
USE_GP_CADJ = os.environ.get("USE_GP_CADJ", "1") == "1"
USE_GP_ATTN = os.environ.get("USE_GP_ATTN", "1") == "1"

# gate permutation for the TimeLSTM: reference order (f, i, o, ct) -> (f, i, ct, o)
PERM1 = [0, 1, 3, 2]
# gate permutation for the day LSTM: reference order (i, f, g, o) -> (i, f, o, g)
PERM2 = [0, 1, 3, 2]


def _rep_ap(tile_ap, reps, inner):
    """AP reading [P, inner] tile as [P, reps, inner] with step-0 repeat."""
    return bass.AP(
        tensor=tile_ap.tensor,
        offset=tile_ap.offset,
        ap=[list(tile_ap.ap[0])] + [[0, reps], [1, inner]],
    )


def build_nc():
    nc = bacc.Bacc()

    def inp(name, shape, dtype=F32):
        return nc.declare_dram_parameter(name, shape, dtype, isOutput=False)

    x_h = inp("x", [EB, 128, TOK], BF16)
    tm1_h = inp("tm1", [1, TOK])
    mask_h = inp("maskbc", [1, TOK], BF16)
    m1_h = inp("m1", [1, TOK], BF16)
    wd_h = inp("wd", [H, H])
    bd_h = inp("bd", [H, 1])
    wall_h = inp("wall", [H, H4], BF16)
    uall_h = inp("uall", [128, EB * H4], BF16)
    bg_h = inp("bg", [H, 4])
    id_h = inp("ident", [128, 128], BF16)
    a1w1_h = inp("a1w1", [H, H], BF16)
    a1b1_h = inp("a1b1", [H, 1])
    a1w2_h = inp("a1w2", [H, H], BF16)
    a1b2_h = inp("a1b2", [H, 1])
    a1v_h = inp("a1vr", [H, 128], BF16)
    l2wih_h = inp("l2wih", [H, H4], BF16)
    l2whh_h = inp("l2whh", [H, H4], BF16)
    bl2_h = inp("bl2", [H, 4])
    a2w1_h = inp("a2w1", [H, H], BF16)
    a2b1_h = inp("a2b1", [H, 1])
    a2w2_h = inp("a2w2", [H, H], BF16)
    a2b2_h = inp("a2b2", [H, 1])
    a2v_h = inp("a2vr", [H, 128], BF16)
    x1w_h = inp("x1w", [H, H])
    x1b_h = inp("x1b", [H, 1])
    x2w_h = inp("x2w", [H, 64])
    x2b_h = inp("x2b", [64, 1])
    sft_h = inp("sft", [17, B])
    h1w_h = inp("h1w", [17, 64])
    h1b_h = inp("h1b", [64, 1])
    h2w_h = inp("h2w", [64, 32])
    h2b_h = inp("h2b", [32, 1])
    hcw0_h = inp("hcw0", [32, S])
    hcws_h = inp("hcws", [4, 128, S])
    hcb_h = inp("hcb", [S, 1])

    out_h = nc.declare_dram_parameter("out", [S, B], F32, isOutput=True)

    # internal DRAM
    cc_in = nc.dram_tensor("cc_in", [64, B], F32)
    cc_out = nc.dram_tensor("cc_out", [S * 64, B], F32, addr_space="Shared")

    with TileContext(nc) as tc:
        with (
            tc.tile_pool(name="big", bufs=1) as big,
            tc.tile_pool(name="wpool", bufs=1) as wp,
            tc.tile_pool(name="state", bufs=1) as st,
            tc.tile_pool(name="xin", bufs=2) as xin,
            tc.tile_pool(name="work", bufs=2) as wk,
            tc.tile_pool(name="ps", bufs=2, space="PSUM") as ps,
        ):
            # ---------------- phase 0: weights + broadcasts -------------
            def wload(h, shape, dtype=F32):
                t = wp.tile(shape, dtype, tag=h.name)
                nc.sync.dma_start(out=t[:, :], in_=h[:, :])
                return t

            wd = wload(wd_h, [H, H])
            bd = wload(bd_h, [H, 1])
            wall = wload(wall_h, [H, H4], BF16)
            uall = wload(uall_h, [128, EB * H4], BF16)
            bg = wload(bg_h, [H, 4])
            ident = wload(id_h, [128, 128], BF16)
            a1w1 = wload(a1w1_h, [H, H], BF16)
            a1b1 = wload(a1b1_h, [H, 1])
            a1w2 = wload(a1w2_h, [H, H], BF16)
            a1b2 = wload(a1b2_h, [H, 1])
            a1vr = wload(a1v_h, [H, 128], BF16)
            l2wih = wload(l2wih_h, [H, H4], BF16)
            l2whh = wload(l2whh_h, [H, H4], BF16)
            bl2 = wload(bl2_h, [H, 4])
            a2w1 = wload(a2w1_h, [H, H], BF16)
            a2b1 = wload(a2b1_h, [H, 1])
            a2w2 = wload(a2w2_h, [H, H], BF16)
            a2b2 = wload(a2b2_h, [H, 1])
            a2vr = wload(a2v_h, [H, 128], BF16)
            x1w = wload(x1w_h, [H, H])
            x1b = wload(x1b_h, [H, 1])
            x2w = wload(x2w_h, [H, 64])
            x2b = wload(x2b_h, [64, 1])
            sft = wload(sft_h, [17, B])
            h1w = wload(h1w_h, [17, 64])
            h1b = wload(h1b_h, [64, 1])
            h2w = wload(h2w_h, [64, 32])
            h2b = wload(h2b_h, [32, 1])
            hcw0 = wload(hcw0_h, [32, S])
            hcb = wload(hcb_h, [S, 1])
            hcws = wp.tile([128, 4 * S], F32, tag="hcws")
            for k in range(4):
                nc.sync.dma_start(out=hcws[:, k * S : (k + 1) * S], in_=hcws_h[k])

            maskbc = big.tile([128, TOK], BF16, tag="maskbc")
            nc.sync.dma_start(
                out=maskbc[:, :], in_=mask_h[0:1, :].partition_broadcast(128)
            )

            tm1bc = big.tile([128, TOK], F32, tag="tm1bc")
            nc.sync.dma_start(
                out=tm1bc[:, :], in_=tm1_h[0:1, :].partition_broadcast(128)
            )
            m1bc = big.tile([128, TOK], BF16, tag="m1bc")
            nc.sync.dma_start(
                out=m1bc[:, :], in_=m1_h[0:1, :].partition_broadcast(128)
            )

            # big persistent buffers
            xu = big.tile([128, 4 * TOK], BF16, tag="xu")
            obuf = big.tile([128, TOK], BF16, tag="obuf")

            # scan state
            h_bf = st.tile([128, N], BF16, tag="h_bf")
            c = st.tile([128, N], F32, tag="c")
            nc.vector.memzero(h_bf[:, :])
            nc.vector.memzero(c[:, :])

            # ------------- phases 1+2: xU production + scan -------------
            def xu_chunk(ci):
                t0 = ci * TPC
                # one consolidated chunk load: xT chunk [128, EB*CH]
                xT = xin.tile([128, EB * CH], BF16, tag="xTc")
                nc.sync.dma_start(
                    out=xT[:, :].rearrange("p (k c) -> p k c", k=EB),
                    in_=x_h[:, :, :].rearrange("k p c -> p k c")[
                        :, :, t0 * N : t0 * N + CH
                    ],
                )
                # matmuls: xu[j][chunk] = sum_k uall[k,j].T @ xT[k][chunk]
                for j in range(4):
                    pt = ps.tile([128, CH], F32, tag="xu")
                    for k in range(EB):
                        nc.tensor.matmul(
                            pt[:, :],
                            uall[:, k * H4 + j * 128 : k * H4 + (j + 1) * 128],
                            xT[:, k * CH : k * CH + CH],
                            start=(k == 0),
                            stop=(k == EB - 1),
                        )
                    # evacuate with gate bias folded in (alternate DVE/ACT)
                    dst = xu[:, j * TOK + t0 * N : j * TOK + t0 * N + CH]
                    if j % 2 == 0:
                        nc.scalar.add(dst, pt[:, :], bg[:, j : j + 1])
                    else:
                        nc.vector.tensor_scalar_add(dst, pt[:, :], bg[:, j : j + 1])

            def scan_step(t):
                sl = slice(t * N, (t + 1) * N)
                # --- c-path first: depends only on c(t-1), overlaps the
                # previous step's h-path tail ---
                gB = ps.tile([128, 2 * N], F32, tag="gB")
                nc.tensor.matmul(
                    gB[:, N : 2 * N], wd[:, :], c[:, :],
                    start=True, stop=True, skip_group_check=True,
                )
                cs1 = st.tile([128, N], F32, tag="cs1")
                nc.scalar.activation(cs1[:, :], gB[:, N : 2 * N], AF.Tanh, bias=bd[:, 0:1])
                # c_adj = c + cs1 * tm1   (gpsimd, off the critical path)
                ca = st.tile([128, N], F32, tag="ca")
                nc.gpsimd.tensor_mul(ca[:, :], cs1[:, :], tm1bc[:, sl])
                nc.gpsimd.tensor_add(ca[:, :], ca[:, :], c[:, :])
                # --- h-path: gate matmuls need h_bf(t-1) ---
                gA = ps.tile([128, 3 * N], F32, tag="gA")
                for j in range(3):  # f, i, ct
                    r = slice(j * N, (j + 1) * N)
                    nc.tensor.matmul(
                        gA[:, r], wall[:, j * 128 : (j + 1) * 128], h_bf[:, :],
                        start=True, stop=False, skip_group_check=True,
                    )
                    nc.tensor.matmul(
                        gA[:, r], ident[:, :], xu[:, j * TOK + t * N : j * TOK + (t + 1) * N],
                        start=False, stop=True, skip_group_check=True,
                    )
                nc.tensor.matmul(
                    gB[:, 0:N], wall[:, 384:512], h_bf[:, :],
                    start=True, stop=False, skip_group_check=True,
                )
                nc.tensor.matmul(
                    gB[:, 0:N], ident[:, :], xu[:, 3 * TOK + t * N : 3 * TOK + (t + 1) * N],
                    start=False, stop=True, skip_group_check=True,
                )
                fic = st.tile([128, 3 * N], F32, tag="fic")
                nc.scalar.activation(fic[:, :], gA[:, :], AF.Sigmoid)
                nc.scalar.activation(obuf[:, sl], gB[:, 0:N], AF.Sigmoid)
                # c = f*c_adj + i*ct   (bv on gpsimd in parallel with av)
                av = st.tile([128, N], F32, tag="av")
                bv = st.tile([128, N], F32, tag="bv")
                nc.gpsimd.tensor_mul(bv[:, :], fic[:, N : 2 * N], fic[:, 2 * N : 3 * N])
                nc.vector.tensor_mul(av[:, :], fic[:, 0:N], ca[:, :])
                nc.vector.tensor_add(c[:, :], av[:, :], bv[:, :])
                tc2 = st.tile([128, N], BF16, tag="tc2")
                nc.scalar.activation(tc2[:, :], c[:, :], AF.Tanh)
                nc.vector.tensor_mul(h_bf[:, :], obuf[:, sl], tc2[:, :])

            for ci in range(NCH):
                xu_chunk(ci)
                for dt_ in range(TPC):
                    scan_step(ci * TPC + dt_)

            # attention-phase buffers (allocated after xT/xin released)
            tmpbig = big.tile([128, TOK], BF16, tag="tmpbig")
            thout = big.tile([128, TOK], BF16, tag="thout")

            # ---------------- phase 3: attention over T -----------------
            # hn = sum_t obuf[:, t, :] * m1
            (nc.gpsimd if USE_GP_ATTN else nc.vector).tensor_mul(tmpbig[:, :], obuf[:, :], m1bc[:, :])
            hn = st.tile([128, N], F32, tag="hn")
            nc.vector.tensor_reduce(
                hn[:, :],
                tmpbig[:, :].rearrange("p (t n) -> p n t", t=T),
                axis=mybir.AxisListType.X,
                op=OP.add,
            )
            hn_bf = st.tile([128, N], BF16, tag="hn_bf")
            nc.vector.tensor_copy(hn_bf[:, :], hn[:, :])
            s1ps = ps.tile([128, N], F32, tag="mm")
            nc.tensor.matmul(s1ps[:, :], a1w1[:, :], hn_bf[:, :], start=True, stop=True)
            s1 = st.tile([128, N], F32, tag="s1")
            nc.scalar.add(s1[:, :], s1ps[:, :], a1b1[:, 0:1])
            # th = tanh(s1 + W2.T @ obuf + b2), in 480-token chunks
            for ci in range(NCH):
                r = slice(ci * CH, (ci + 1) * CH)
                sp = ps.tile([128, CH], F32, tag="mm")
                nc.tensor.matmul(sp[:, :], a1w2[:, :], obuf[:, r], start=True, stop=True)
                ti = wk.tile([128, CH], F32, tag="ti")
                nc.vector.tensor_add(
                    ti[:, :].rearrange("p (r n) -> p r n", r=TPC),
                    sp[:, :].rearrange("p (r n) -> p r n", r=TPC),
                    _rep_ap(s1[:, :], TPC, N),
                )
                nc.scalar.activation(thout[:, r], ti[:, :], AF.Tanh, bias=a1b2[:, 0:1])
            # scores replicated across partitions: lhsT = V tiled into all 128
            # columns, so out[p, tok] = sum_j V[j] th[j, tok] for every p.
            # The whole softmax then stays lane-local (no partition broadcast).
            neg30 = st.tile([128, 1], F32, tag="neg30")
            nc.vector.memset(neg30[:, :], -30.0)
            ewbc = big.tile([128, TOK], BF16, tag="m1bc")
            for ci in range(NCH):
                r = slice(ci * CH, (ci + 1) * CH)
                scp = ps.tile([128, CH], F32, tag="mm")
                nc.tensor.matmul(scp[:, :], a1vr[:, :], thout[:, r], start=True, stop=True)
                smc = wk.tile([128, CH], F32, tag="ti")
                nc.vector.scalar_tensor_tensor(
                    smc[:, :], scp[:, :], 30.0, maskbc[:, r], OP.add, OP.mult
                )
                nc.scalar.activation(ewbc[:, r], smc[:, :], AF.Exp, bias=neg30[:, 0:1])
            zr = st.tile([128, N], F32, tag="zr")
            nc.vector.tensor_reduce(
                zr[:, :],
                ewbc[:, :].rearrange("p (t n) -> p n t", t=T),
                axis=mybir.AxisListType.X,
                op=OP.add,
            )
            rz = st.tile([128, N], F32, tag="rz")
            nc.vector.reciprocal(rz[:, :], zr[:, :])
            nc.vector.tensor_mul(tmpbig[:, :], obuf[:, :], ewbc[:, :])
            ctxr = st.tile([128, N], F32, tag="ctxr")
            nc.vector.tensor_reduce(
                ctxr[:, :],
                tmpbig[:, :].rearrange("p (t n) -> p n t", t=T),
                axis=mybir.AxisListType.X,
                op=OP.add,
            )
            ctx_bf = st.tile([128, N], BF16, tag="ctx_bf")
            nc.vector.tensor_mul(ctx_bf[:, :], ctxr[:, :], rz[:, :])

            # ---------------- phase 4: day LSTM (D steps) ---------------
            hs_bf = st.tile([128, N], BF16, tag="hs_bf")
            h2st = st.tile([128, B], BF16, tag="h2st")
            c2st = st.tile([128, B], F32, tag="c2st")
            nc.vector.memzero(h2st[:, :])
            nc.vector.memzero(c2st[:, :])
            for d in range(D):
                xin_d = ctx_bf[:, :].rearrange("p (b d) -> p d b", d=D)[:, d, :]
                g2 = ps.tile([128, 4 * B], F32, tag="mm")
                for j in range(4):
                    r = slice(j * B, (j + 1) * B)
                    nc.tensor.matmul(
                        g2[:, r], l2wih[:, j * 128 : (j + 1) * 128], xin_d,
                        start=True, stop=False, skip_group_check=True,
                    )
                    nc.tensor.matmul(
                        g2[:, r], l2whh[:, j * 128 : (j + 1) * 128], h2st[:, :],
                        start=False, stop=True, skip_group_check=True,
                    )
                sg = st.tile([128, 3 * B], F32, tag="sg")
                for j in range(3):  # i, f, o
                    nc.scalar.activation(
                        sg[:, j * B : (j + 1) * B], g2[:, j * B : (j + 1) * B],
                        AF.Sigmoid, bias=bl2[:, j : j + 1],
                    )
                tg = st.tile([128, B], F32, tag="tg")
                nc.scalar.activation(tg[:, :], g2[:, 3 * B : 4 * B], AF.Tanh, bias=bl2[:, 3:4])
                a2v_ = st.tile([128, B], F32, tag="a2v_")
                b2v_ = st.tile([128, B], F32, tag="b2v_")
                nc.vector.tensor_mul(a2v_[:, :], sg[:, B : 2 * B], c2st[:, :])
                nc.vector.tensor_mul(b2v_[:, :], sg[:, 0:B], tg[:, :])
                nc.vector.tensor_add(c2st[:, :], a2v_[:, :], b2v_[:, :])
                tc2b = st.tile([128, B], BF16, tag="tc2b")
                nc.scalar.activation(tc2b[:, :], c2st[:, :], AF.Tanh)
                nc.vector.tensor_mul(h2st[:, :], sg[:, 2 * B : 3 * B], tc2b[:, :])
                nc.vector.tensor_copy(
                    hs_bf[:, :].rearrange("p (b d) -> p d b", d=D)[:, d, :], h2st[:, :]
                )

            # ---------------- phase 5: attention over D -----------------
            s1aps = ps.tile([128, B], F32, tag="mm")
            nc.tensor.matmul(s1aps[:, :], a2w1[:, :], h2st[:, :], start=True, stop=True)
            s1a = st.tile([128, B], F32, tag="s1a")
            nc.scalar.add(s1a[:, :], s1aps[:, :], a2b1[:, 0:1])
            s2aps = ps.tile([128, N], F32, tag="mm")
            nc.tensor.matmul(s2aps[:, :], a2w2[:, :], hs_bf[:, :], start=True, stop=True)
            t2i = st.tile([128, N], F32, tag="t2i")
            # hs layout is (b, d): s1a must repeat per-b along d -> use [b][d] view
            nc.vector.tensor_add(
                t2i[:, :].rearrange("p (b d) -> p b d", d=D),
                s2aps[:, :].rearrange("p (b d) -> p b d", d=D),
                bass.AP(
                    tensor=s1a.tensor,
                    offset=s1a[:, :].offset,
                    ap=[list(s1a[:, :].ap[0])] + [[1, B], [0, D]],
                ),
            )
            th2 = st.tile([128, N], BF16, tag="th2")
            nc.scalar.activation(th2[:, :], t2i[:, :], AF.Tanh, bias=a2b2[:, 0:1])
            # replicated scores again: out[p, (b,d)] = sum_j V2[j] th2[j, (b,d)]
            sc2p = ps.tile([128, N], F32, tag="mm")
            nc.tensor.matmul(sc2p[:, :], a2vr[:, :], th2[:, :], start=True, stop=True)
            ew2r = st.tile([128, N], BF16, tag="ew2r")
            nc.scalar.activation(ew2r[:, :], sc2p[:, :], AF.Exp)
            z2r = st.tile([128, B], F32, tag="z2r")
            nc.vector.tensor_reduce(
                z2r[:, :],
                ew2r[:, :].rearrange("p (b d) -> p b d", d=D),
                axis=mybir.AxisListType.X,
                op=OP.add,
            )
            rz2 = st.tile([128, B], F32, tag="rz2")
            nc.vector.reciprocal(rz2[:, :], z2r[:, :])
            tmp2 = st.tile([128, N], BF16, tag="tmp2")
            nc.vector.tensor_mul(tmp2[:, :], hs_bf[:, :], ew2r[:, :])
            ctx2r = st.tile([128, B], F32, tag="ctx2r")
            nc.vector.tensor_reduce(
                ctx2r[:, :],
                tmp2[:, :].rearrange("p (b d) -> p b d", d=D),
                axis=mybir.AxisListType.X,
                op=OP.add,
            )
            ctx2 = st.tile([128, B], F32, tag="ctx2")
            nc.vector.tensor_mul(ctx2[:, :], ctx2r[:, :], rz2[:, :])

            # ---------------- phase 6: per-stock head + global ----------
            y1ps = ps.tile([128, B], F32, tag="mm")
            nc.tensor.matmul(y1ps[:, :], x1w[:, :], ctx2[:, :], start=True, stop=True)
            y1 = st.tile([128, B], F32, tag="y1")
            nc.scalar.activation(y1[:, :], y1ps[:, :], AF.Relu, bias=x1b[:, 0:1])
            o2ps = ps.tile([64, B], F32, tag="mm")
            nc.tensor.matmul(o2ps[:, :], x2w[:, :], y1[:, :], start=True, stop=True)
            txt = st.tile([64, B], F32, tag="txt")
            nc.scalar.add(txt[:, :], o2ps[:, :], x2b[:, 0:1])
            nc.sync.dma_start(out=cc_in[:, :], in_=txt[:, :])
            nc.gpsimd.collective_compute(
                "AllGather",
                OP.bypass,
                replica_groups=[list(range(NCORES))],
                ins=[cc_in[:, :]],
                outs=[cc_out[:, :]],
            )
            # xs path
            y2ps = ps.tile([64, B], F32, tag="mm")
            nc.tensor.matmul(y2ps[:, :], h1w[:, :], sft[:, :], start=True, stop=True)
            y2 = st.tile([64, B], F32, tag="y2")
            nc.scalar.activation(y2[:, :], y2ps[:, :], AF.Relu, bias=h1b[:, 0:1])
            xsps = ps.tile([32, B], F32, tag="mm")
            nc.tensor.matmul(xsps[:, :], h2w[:, :], y2[:, :], start=True, stop=True)
            xst = st.tile([32, B], F32, tag="xst")
            nc.scalar.add(xst[:, :], xsps[:, :], h2b[:, 0:1])
            # final: out.T = tanh(hc_W.T @ [xs; text].T + hc_b)
            ga = st.tile([128, 4 * B], F32, tag="ga")
            for k in range(4):
                nc.sync.dma_start(
                    out=ga[:, k * B : (k + 1) * B], in_=cc_out[k * 128 : (k + 1) * 128, :]
                )
            fps = ps.tile([S, B], F32, tag="mm")
            nc.tensor.matmul(fps[:, :], hcw0[:, :], xst[:, :], start=True, stop=False)
            for k in range(4):
                nc.tensor.matmul(
                    fps[:, :], hcws[:, k * S : (k + 1) * S], ga[:, k * B : (k + 1) * B],
                    start=False, stop=(k == 3),
                )
            osb = st.tile([S, B], F32, tag="osb")
            nc.scalar.activation(osb[:, :], fps[:, :], AF.Tanh, bias=hcb[:, 0:1])
            nc.sync.dma_start(out=out_h[:, :], in_=osb[:, :])

    return nc


def make_in_maps(
    stock_feats, sentence_feat, time_feats, len_tweets,
    tl_Wall, tl_ball, tl_Uall, tl_bU, tl_Wd, tl_bd,
    a1_W1, a1_b1, a1_W2, a1_b2, a1_V, a1_bV,
    l2_Wih, l2_bih, l2_Whh, l2_bhh,
    a2_W1, a2_b1, a2_W2, a2_b2, a2_V, a2_bV,
    x1_W, x1_b, x2_W, x2_b,
    h1_W, h1_b, h2_W, h2_b, hc_W, hc_b,
):
    f32 = np.float32

    def permcols(w, perm):
        # w [..., 4*128] -> permuted gate blocks
        shp = w.shape
        wr = w.reshape(shp[:-1] + (4, 128))
        return wr[..., perm, :].reshape(shp)

    in_maps = []
    shared = {}
    shared["sft"] = np.ascontiguousarray(stock_feats.T).astype(f32)
    shared["h1w"] = np.asarray(h1_W, f32)
    shared["h1b"] = np.asarray(h1_b, f32).reshape(64, 1)
    shared["h2w"] = np.asarray(h2_W, f32)
    shared["h2b"] = np.asarray(h2_b, f32).reshape(32, 1)
    shared["hcw0"] = np.asarray(hc_W, f32)[:32]
    shared["hcws"] = np.ascontiguousarray(
        np.asarray(hc_W, f32)[32:].reshape(4, 128, S)
    )
    shared["hcb"] = np.asarray(hc_b, f32).reshape(S, 1)
    shared["ident"] = np.eye(128, dtype=f32).astype(BF)

    for s in range(S):
        m = dict(shared)
        xs = np.asarray(sentence_feat[:, s], f32)          # [B, D, T, E]
        xbf = xs.astype(BF)                                # cast first (cheap)
        # [B, D, T, E] -> [E, T, B, D] -> [EB, 128, T*N]
        m["x"] = np.ascontiguousarray(xbf.transpose(3, 2, 0, 1)).reshape(EB, 128, TOK)
        tt = np.asarray(time_feats[:, s], f32)             # [B, D, T]
        m["tm1"] = (
            np.ascontiguousarray(tt.transpose(2, 0, 1)).reshape(1, TOK) - 1.0
        ).astype(f32)
        lens = np.asarray(len_tweets[:, s]).reshape(N)     # [N] int
        tgrid = np.arange(T)[:, None]
        m["maskbc"] = (tgrid < lens[None, :]).astype(f32).reshape(1, TOK).astype(BF)
        m["m1"] = (tgrid == (lens[None, :] - 1)).astype(f32).reshape(1, TOK).astype(BF)
        m["wd"] = np.asarray(tl_Wd[s], f32)
        m["bd"] = np.asarray(tl_bd[s], f32).reshape(H, 1)
        m["wall"] = permcols(np.asarray(tl_Wall[s], f32), PERM1).astype(BF)
        u = permcols(np.asarray(tl_Uall[s], f32), PERM1)   # [E, 512]
        m["uall"] = np.ascontiguousarray(
            u.reshape(EB, 128, H4).transpose(1, 0, 2)
        ).reshape(128, EB * H4).astype(BF)
        bgv = permcols(
            (np.asarray(tl_ball[s], f32) + np.asarray(tl_bU[s], f32))[None, :], PERM1
        )[0]
        m["bg"] = np.ascontiguousarray(bgv.reshape(4, 128).T).astype(f32)
        m["a1w1"] = np.asarray(a1_W1[s], f32).astype(BF)
        m["a1b1"] = np.asarray(a1_b1[s], f32).reshape(H, 1)
        m["a1w2"] = np.asarray(a1_W2[s], f32).astype(BF)
        m["a1b2"] = np.asarray(a1_b2[s], f32).reshape(H, 1)
        m["a1vr"] = np.tile(np.asarray(a1_V[s], f32).reshape(H, 1), (1, 128)).astype(BF)
        m["l2wih"] = permcols(np.asarray(l2_Wih[s], f32), PERM2).astype(BF)
        m["l2whh"] = permcols(np.asarray(l2_Whh[s], f32), PERM2).astype(BF)
        bl2v = permcols(
            (np.asarray(l2_bih[s], f32) + np.asarray(l2_bhh[s], f32))[None, :], PERM2
        )[0]
        m["bl2"] = np.ascontiguousarray(bl2v.reshape(4, 128).T).astype(f32)
        m["a2w1"] = np.asarray(a2_W1[s], f32).astype(BF)
        m["a2b1"] = np.asarray(a2_b1[s], f32).reshape(H, 1)
        m["a2w2"] = np.asarray(a2_W2[s], f32).astype(BF)
        m["a2b2"] = np.asarray(a2_b2[s], f32).reshape(H, 1)
        m["a2vr"] = np.tile(np.asarray(a2_V[s], f32).reshape(H, 1), (1, 128)).astype(BF)
        m["x1w"] = np.asarray(x1_W[s], f32)
        m["x1b"] = np.asarray(x1_b[s], f32).reshape(H, 1)
        m["x2w"] = np.asarray(x2_W[s], f32)
        m["x2b"] = np.asarray(x2_b[s], f32).reshape(64, 1)
        in_maps.append(m)
    return in_maps


_CACHED_NC = None
TRACE = False
LAST_EXEC_NS = None
LAST_RESULT = None


def kernel(**inputs) -> np.ndarray:
    global _CACHED_NC, LAST_EXEC_NS, LAST_RESULT
    from concourse.bass_utils import run_bass_kernel_spmd

    in_maps = make_in_maps(**inputs)
    if _CACHED_NC is None:
        nc = build_nc()
        nc.finalize()
        _CACHED_NC = nc
    res = run_bass_kernel_spmd(
        _CACHED_NC, in_maps, list(range(NCORES)), trace=TRACE
    )
    LAST_EXEC_NS = res.exec_time_ns
    LAST_RESULT = res
    out_t = res.results[0]["out"]          # [S, B]
    return np.ascontiguousarray(out_t.T).astype(np.float32)  # [B, S]


# revision 25
# speedup vs baseline: 1.1364x; 1.1364x over previous
"""Trainium2 Bass kernel for nn_Actor_73057393705109.

Architecture (per stock s, sharded one stock per NeuronCore, 8 cores):
  TimeLSTM over T=30 steps of B*D=160 sequences (E=768 -> H=128)
  -> masked attention over T -> day-LSTM over D=5 -> attention over D
  -> 2-layer MLP head per stock -> AllGather -> global linear head.

Device layout: "transposed" everywhere — feature dims on SBUF partitions,
sequence index n = b*D + d on the free dim. Matmul operands in bf16
(1 cyc/row on the PE), state and softmax math in fp32.
"""

import sys

if "/opt/trn_rl_repo" not in sys.path:
    sys.path.insert(0, "/opt/trn_rl_repo")

import ml_dtypes
import numpy as np

import concourse.bacc as bacc
import concourse.bass as bass
import concourse.mybir as mybir
from concourse import library_config
from concourse.tile import TileContext

F32 = mybir.dt.float32
BF16 = mybir.dt.bfloat16
AF = mybir.ActivationFunctionType
OP = mybir.AluOpType
BF = ml_dtypes.bfloat16

S, B, D, T, E, H = 8, 32, 5, 30, 768, 128
H4 = 4 * H
N = B * D            # 160 sequences per stock
TOK = T * N          # 4800 tokens, t-major: tok = t*N + n
EB = E // 128        # 6 e-blocks
TPC = 3              # t-steps per xU chunk
CH = TPC * N         # 480 tokens per chunk
NCH = T // TPC       # 10 chunks
NCORES = 8
import os
USE_GP_CADJ = os.environ.get("USE_GP_CADJ", "1") == "1"
USE_GP_ATTN = os.environ.get("USE_GP_ATTN", "1") == "1"


# packed weight layout: (name, rows, cols) concatenated along the free dim
W_BF = [("wall", 128, H4), ("uall", 128, EB * H4), ("ident", 128, 128),
        ("a1w1", 128, H), ("a1w2", 128, H), ("a1vr", 128, 128),
        ("l2wih", 128, H4), ("l2whh", 128, H4),
        ("a2w1", 128, H), ("a2w2", 128, H), ("a2vr", 128, 128)]
W_F32 = [("wd", 128, H), ("bd", 128, 1), ("bg", 128, 4),
         ("a1b1", 128, 1), ("a1b2", 128, 1), ("bl2", 128, 4),
         ("a2b1", 128, 1), ("a2b2", 128, 1),
         ("x1w", 128, H), ("x1b", 128, 1), ("x2w", 128, 64), ("x2b", 64, 1),
         ("sft", 17, B), ("h1w", 17, 64), ("h1b", 64, 1),
         ("h2w", 64, 32), ("h2b", 32, 1), ("hcw0", 32, S),
         ("hcb", 8, 1), ("hcws", 128, 4 * S)]
WBF_COLS = sum(c for _, _, c in W_BF)
WF32_COLS = sum(c for _, _, c in W_F32)

# gate permutation for the TimeLSTM: reference order (f, i, o, ct) -> (f, i, ct, o)
PERM1 = [0, 1, 3, 2]
# gate permutation for the day LSTM: reference order (i, f, g, o) -> (i, f, o, g)
PERM2 = [0, 1, 3, 2]


def _rep_ap(tile_ap, reps, inner):
    """AP reading [P, inner] tile as [P, reps, inner] with step-0 repeat."""
    return bass.AP(
        tensor=tile_ap.tensor,
        offset=tile_ap.offset,
        ap=[list(tile_ap.ap[0])] + [[0, reps], [1, inner]],
    )


def build_nc():
    nc = bacc.Bacc()

    def inp(name, shape, dtype=F32):
        return nc.declare_dram_parameter(name, shape, dtype, isOutput=False)

    x_h = inp("x", [EB, 128, TOK], BF16)
    tm1_h = inp("tm1", [1, TOK])
    mask_h = inp("maskbc", [1, TOK], BF16)
    m1_h = inp("m1", [1, TOK], BF16)
    wbf_h = inp("wbf", [128, WBF_COLS], BF16)
    wf32_h = inp("wf32", [128, WF32_COLS])

    out_h = nc.declare_dram_parameter("out", [S, B], F32, isOutput=True)

    # internal DRAM
    cc_in = nc.dram_tensor("cc_in", [64, B], F32)
    cc_out = nc.dram_tensor("cc_out", [S * 64, B], F32, addr_space="Shared")

    with TileContext(nc) as tc:
        with (
            tc.tile_pool(name="big", bufs=1) as big,
            tc.tile_pool(name="wpool", bufs=1) as wp,
            tc.tile_pool(name="state", bufs=1) as st,
            tc.tile_pool(name="xin", bufs=2) as xin,
            tc.tile_pool(name="work", bufs=2) as wk,
            tc.tile_pool(name="ps", bufs=2, space="PSUM") as ps,
        ):
            # ---------------- phase 0: weights (2 packed DMAs) ----------
            wbf_t = wp.tile([128, WBF_COLS], BF16, tag="wbf")
            nc.sync.dma_start(out=wbf_t[:, :], in_=wbf_h[:, :])
            wf32_t = wp.tile([128, WF32_COLS], F32, tag="wf32")
            nc.sync.dma_start(out=wf32_t[:, :], in_=wf32_h[:, :])

            def _mk_slices(table, tile):
                out, off = {}, 0
                for nm, rows, cols in table:
                    out[nm] = tile[0:rows, off : off + cols]
                    off += cols
                return out

            wsl = _mk_slices(W_BF, wbf_t)
            wsl.update(_mk_slices(W_F32, wf32_t))
            wall, uall, ident = wsl["wall"], wsl["uall"], wsl["ident"]
            a1w1, a1w2, a1vr = wsl["a1w1"], wsl["a1w2"], wsl["a1vr"]
            l2wih, l2whh = wsl["l2wih"], wsl["l2whh"]
            a2w1, a2w2, a2vr = wsl["a2w1"], wsl["a2w2"], wsl["a2vr"]
            wd, bd, bg, bl2 = wsl["wd"], wsl["bd"], wsl["bg"], wsl["bl2"]
            a1b1, a1b2, a2b1, a2b2 = wsl["a1b1"], wsl["a1b2"], wsl["a2b1"], wsl["a2b2"]
            x1w, x1b, x2w, x2b = wsl["x1w"], wsl["x1b"], wsl["x2w"], wsl["x2b"]
            sft, h1w, h1b = wsl["sft"], wsl["h1w"], wsl["h1b"]
            h2w, h2b, hcw0, hcb, hcws = wsl["h2w"], wsl["h2b"], wsl["hcw0"], wsl["hcb"], wsl["hcws"]

            maskbc = big.tile([128, TOK], BF16, tag="maskbc")
            tm1bc = big.tile([128, TOK], F32, tag="tm1bc")
            m1bc = big.tile([128, TOK], BF16, tag="m1bc")

            def tm1_load(ci):
                r = slice(ci * CH, (ci + 1) * CH)
                nc.scalar.dma_start(
                    out=tm1bc[:, r], in_=tm1_h[0:1, r].partition_broadcast(128)
                )

            # big persistent buffers
            xu = big.tile([128, 4 * TOK], BF16, tag="xu")
            obuf = big.tile([128, TOK], BF16, tag="obuf")

            # scan state
            h_bf = st.tile([128, N], BF16, tag="h_bf")
            c = st.tile([128, N], F32, tag="c")
            nc.vector.memzero(h_bf[:, :])
            nc.vector.memzero(c[:, :])

            # ------------- phases 1+2: xU production + scan -------------
            def xu_load(ci):
                t0 = ci * TPC
                # one consolidated chunk load: xT chunk [128, EB*CH]
                xT = xin.tile([128, EB * CH], BF16, tag="xTc")
                nc.sync.dma_start(
                    out=xT[:, :].rearrange("p (k c) -> p k c", k=EB),
                    in_=x_h[:, :, :].rearrange("k p c -> p k c")[
                        :, :, t0 * N : t0 * N + CH
                    ],
                )
                return xT

            def xu_j(ci, xT, j):
                # xu[j][chunk] = sum_k uall[k,j].T @ xT[k][chunk], bias folded
                t0 = ci * TPC
                pt = ps.tile([128, CH], F32, tag="xu")
                for k in range(EB):
                    nc.tensor.matmul(
                        pt[:, :],
                        uall[:, k * H4 + j * 128 : k * H4 + (j + 1) * 128],
                        xT[:, k * CH : k * CH + CH],
                        start=(k == 0),
                        stop=(k == EB - 1),
                    )
                dst = xu[:, j * TOK + t0 * N : j * TOK + t0 * N + CH]
                if j % 2 == 0:
                    nc.scalar.add(dst, pt[:, :], bg[:, j : j + 1])
                else:
                    nc.vector.tensor_scalar_add(dst, pt[:, :], bg[:, j : j + 1])

            def xu_chunk(ci):
                xT = xu_load(ci)
                for j in range(4):
                    xu_j(ci, xT, j)

            def scan_step(t):
                sl = slice(t * N, (t + 1) * N)
                # --- c-path first: depends only on c(t-1), overlaps the
                # previous step's h-path tail ---
                gB = ps.tile([128, 2 * N], F32, tag="gB")
                nc.tensor.matmul(
                    gB[:, N : 2 * N], wd[:, :], c[:, :],
                    start=True, stop=True, skip_group_check=True,
                )
                cs1 = st.tile([128, N], F32, tag="cs1")
                nc.scalar.activation(cs1[:, :], gB[:, N : 2 * N], AF.Tanh, bias=bd[:, 0:1])
                # c_adj = c + cs1 * tm1   (gpsimd, off the critical path)
                ca = st.tile([128, N], F32, tag="ca")
                nc.gpsimd.tensor_mul(ca[:, :], cs1[:, :], tm1bc[:, sl])
                nc.gpsimd.tensor_add(ca[:, :], ca[:, :], c[:, :])
                # --- h-path: gate matmuls need h_bf(t-1) ---
                gA = ps.tile([128, 3 * N], F32, tag="gA")
                for j in range(3):  # f, i, ct
                    r = slice(j * N, (j + 1) * N)
                    nc.tensor.matmul(
                        gA[:, r], wall[:, j * 128 : (j + 1) * 128], h_bf[:, :],
                        start=True, stop=False, skip_group_check=True,
                    )
                    nc.tensor.matmul(
                        gA[:, r], ident[:, :], xu[:, j * TOK + t * N : j * TOK + (t + 1) * N],
                        start=False, stop=True, skip_group_check=True,
                    )
                nc.tensor.matmul(
                    gB[:, 0:N], wall[:, 384:512], h_bf[:, :],
                    start=True, stop=False, skip_group_check=True,
                )
                nc.tensor.matmul(
                    gB[:, 0:N], ident[:, :], xu[:, 3 * TOK + t * N : 3 * TOK + (t + 1) * N],
                    start=False, stop=True, skip_group_check=True,
                )
                fic = st.tile([128, 3 * N], F32, tag="fic")
                nc.scalar.activation(fic[:, :], gA[:, :], AF.Sigmoid)
                nc.scalar.activation(obuf[:, sl], gB[:, 0:N], AF.Sigmoid)
                # c = f*c_adj + i*ct   (bv on gpsimd in parallel with av)
                av = st.tile([128, N], F32, tag="av")
                bv = st.tile([128, N], F32, tag="bv")
                nc.gpsimd.tensor_mul(bv[:, :], fic[:, N : 2 * N], fic[:, 2 * N : 3 * N])
                nc.vector.tensor_mul(av[:, :], fic[:, 0:N], ca[:, :])
                nc.vector.tensor_add(c[:, :], av[:, :], bv[:, :])
                tc2 = st.tile([128, N], BF16, tag="tc2")
                nc.scalar.activation(tc2[:, :], c[:, :], AF.Tanh)
                nc.vector.tensor_mul(h_bf[:, :], obuf[:, sl], tc2[:, :])

            # Fine-grained static interleave: the PE is in-order, so xU
            # matmuls for chunk ci+2 are woven between scan steps — while a
            # scan step waits on h_bf, the queued xU matmuls are NOT stuck
            # behind it across chunk boundaries (prefetch distance 2 keeps
            # every scan step's xu slice ready well in advance).
            tm1_load(0)
            tm1_load(1)
            xu_chunk(0)
            xu_chunk(1)
            pre_xT = {}
            for ci in range(NCH):
                for dt_ in range(TPC):
                    scan_step(ci * TPC + dt_)
                    nxt = ci + 2
                    if nxt < NCH:
                        if dt_ == 0:
                            tm1_load(nxt)
                            pre_xT[nxt] = xu_load(nxt)
                            xu_j(nxt, pre_xT[nxt], 0)
                        elif dt_ == 1:
                            xu_j(nxt, pre_xT[nxt], 1)
                            xu_j(nxt, pre_xT[nxt], 2)
                        else:
                            xu_j(nxt, pre_xT.pop(nxt), 3)

            if os.environ.get("SKIP_TAIL", "0") == "1":
                osb0 = st.tile([S, B], F32, tag="osb0")
                nc.vector.tensor_copy(osb0[:, :], obuf[0:S, 0:B])
                nc.sync.dma_start(out=out_h[:, :], in_=osb0[:, :])
                return nc
            # attention-phase buffers (allocated after xT/xin released)
            nc.sync.dma_start(
                out=m1bc[:, :], in_=m1_h[0:1, :].partition_broadcast(128)
            )
            nc.scalar.dma_start(
                out=maskbc[:, :], in_=mask_h[0:1, :].partition_broadcast(128)
            )
            tmpbig = big.tile([128, TOK], BF16, tag="tmpbig")
            thout = big.tile([128, TOK], BF16, tag="thout")

            # ---------------- phase 3: attention over T -----------------
            # hn = sum_t obuf[:, t, :] * m1
            nc.vector.tensor_mul(tmpbig[:, :], obuf[:, :], m1bc[:, :])
            hn = st.tile([128, N], F32, tag="hn")
            nc.vector.tensor_reduce(
                hn[:, :],
                tmpbig[:, :].rearrange("p (t n) -> p n t", t=T),
                axis=mybir.AxisListType.X,
                op=OP.add,
            )
            hn_bf = st.tile([128, N], BF16, tag="hn_bf")
            nc.vector.tensor_copy(hn_bf[:, :], hn[:, :])
            s1ps = ps.tile([128, N], F32, tag="mm")
            nc.tensor.matmul(s1ps[:, :], a1w1[:, :], hn_bf[:, :], start=True, stop=True)
            s1 = st.tile([128, N], F32, tag="s1")
            nc.scalar.add(s1[:, :], s1ps[:, :], a1b1[:, 0:1])
            # th = tanh(s1 + W2.T @ obuf + b2), in 480-token chunks
            for ci in range(NCH):
                r = slice(ci * CH, (ci + 1) * CH)
                sp = ps.tile([128, CH], F32, tag="mm")
                nc.tensor.matmul(sp[:, :], a1w2[:, :], obuf[:, r], start=True, stop=True)
                ti = wk.tile([128, CH], F32, tag="ti")
                nc.vector.tensor_add(
                    ti[:, :].rearrange("p (r n) -> p r n", r=TPC),
                    sp[:, :].rearrange("p (r n) -> p r n", r=TPC),
                    _rep_ap(s1[:, :], TPC, N),
                )
                nc.scalar.activation(thout[:, r], ti[:, :], AF.Tanh, bias=a1b2[:, 0:1])
            # scores replicated across partitions: lhsT = V tiled into all 128
            # columns, so out[p, tok] = sum_j V[j] th[j, tok] for every p.
            # The whole softmax then stays lane-local (no partition broadcast).
            neg30 = st.tile([128, 1], F32, tag="neg30")
            nc.vector.memset(neg30[:, :], -30.0)
            ewbc = big.tile([128, TOK], BF16, tag="m1bc")
            for ci in range(NCH):
                r = slice(ci * CH, (ci + 1) * CH)
                scp = ps.tile([128, CH], F32, tag="mm")
                nc.tensor.matmul(scp[:, :], a1vr[:, :], thout[:, r], start=True, stop=True)
                smc = wk.tile([128, CH], F32, tag="ti")
                nc.vector.scalar_tensor_tensor(
                    smc[:, :], scp[:, :], 30.0, maskbc[:, r], OP.add, OP.mult
                )
                nc.scalar.activation(ewbc[:, r], smc[:, :], AF.Exp, bias=neg30[:, 0:1])
            zr = st.tile([128, N], F32, tag="zr")
            nc.vector.tensor_reduce(
                zr[:, :],
                ewbc[:, :].rearrange("p (t n) -> p n t", t=T),
                axis=mybir.AxisListType.X,
                op=OP.add,
            )
            rz = st.tile([128, N], F32, tag="rz")
            nc.vector.reciprocal(rz[:, :], zr[:, :])
            nc.vector.tensor_mul(tmpbig[:, :], obuf[:, :], ewbc[:, :])
            ctxr = st.tile([128, N], F32, tag="ctxr")
            nc.vector.tensor_reduce(
                ctxr[:, :],
                tmpbig[:, :].rearrange("p (t n) -> p n t", t=T),
                axis=mybir.AxisListType.X,
                op=OP.add,
            )
            ctx_bf = st.tile([128, N], BF16, tag="ctx_bf")
            nc.vector.tensor_mul(ctx_bf[:, :], ctxr[:, :], rz[:, :])

            # ---------------- phase 4: day LSTM (D steps) ---------------
            hs_bf = st.tile([128, N], BF16, tag="hs_bf")
            h2st = st.tile([128, B], BF16, tag="h2st")
            c2st = st.tile([128, B], F32, tag="c2st")
            nc.vector.memzero(h2st[:, :])
            nc.vector.memzero(c2st[:, :])
            for d in range(D):
                xin_d = ctx_bf[:, :].rearrange("p (b d) -> p d b", d=D)[:, d, :]
                g2 = ps.tile([128, 4 * B], F32, tag="mm")
                for j in range(4):
                    r = slice(j * B, (j + 1) * B)
                    nc.tensor.matmul(
                        g2[:, r], l2wih[:, j * 128 : (j + 1) * 128], xin_d,
                        start=True, stop=False, skip_group_check=True,
                    )
                    nc.tensor.matmul(
                        g2[:, r], l2whh[:, j * 128 : (j + 1) * 128], h2st[:, :],
                        start=False, stop=True, skip_group_check=True,
                    )
                sg = st.tile([128, 3 * B], F32, tag="sg")
                for j in range(3):  # i, f, o
                    nc.scalar.activation(
                        sg[:, j * B : (j + 1) * B], g2[:, j * B : (j + 1) * B],
                        AF.Sigmoid, bias=bl2[:, j : j + 1],
                    )
                tg = st.tile([128, B], F32, tag="tg")
                nc.scalar.activation(tg[:, :], g2[:, 3 * B : 4 * B], AF.Tanh, bias=bl2[:, 3:4])
                a2v_ = st.tile([128, B], F32, tag="a2v_")
                b2v_ = st.tile([128, B], F32, tag="b2v_")
                nc.vector.tensor_mul(a2v_[:, :], sg[:, B : 2 * B], c2st[:, :])
                nc.vector.tensor_mul(b2v_[:, :], sg[:, 0:B], tg[:, :])
                nc.vector.tensor_add(c2st[:, :], a2v_[:, :], b2v_[:, :])
                tc2b = st.tile([128, B], BF16, tag="tc2b")
                nc.scalar.activation(tc2b[:, :], c2st[:, :], AF.Tanh)
                nc.vector.tensor_mul(h2st[:, :], sg[:, 2 * B : 3 * B], tc2b[:, :])
                nc.vector.tensor_copy(
                    hs_bf[:, :].rearrange("p (b d) -> p d b", d=D)[:, d, :], h2st[:, :]
                )

            # ---------------- phase 5: attention over D -----------------
            s1aps = ps.tile([128, B], F32, tag="mm")
            nc.tensor.matmul(s1aps[:, :], a2w1[:, :], h2st[:, :], start=True, stop=True)
            s1a = st.tile([128, B], F32, tag="s1a")
            nc.scalar.add(s1a[:, :], s1aps[:, :], a2b1[:, 0:1])
            s2aps = ps.tile([128, N], F32, tag="mm")
            nc.tensor.matmul(s2aps[:, :], a2w2[:, :], hs_bf[:, :], start=True, stop=True)
            t2i = st.tile([128, N], F32, tag="t2i")
            # hs layout is (b, d): s1a must repeat per-b along d -> use [b][d] view
            nc.vector.tensor_add(
                t2i[:, :].rearrange("p (b d) -> p b d", d=D),
                s2aps[:, :].rearrange("p (b d) -> p b d", d=D),
                bass.AP(
                    tensor=s1a.tensor,
                    offset=s1a[:, :].offset,
                    ap=[list(s1a[:, :].ap[0])] + [[1, B], [0, D]],
                ),
            )
            th2 = st.tile([128, N], BF16, tag="th2")
            nc.scalar.activation(th2[:, :], t2i[:, :], AF.Tanh, bias=a2b2[:, 0:1])
            # replicated scores again: out[p, (b,d)] = sum_j V2[j] th2[j, (b,d)]
            sc2p = ps.tile([128, N], F32, tag="mm")
            nc.tensor.matmul(sc2p[:, :], a2vr[:, :], th2[:, :], start=True, stop=True)
            ew2r = st.tile([128, N], BF16, tag="ew2r")
            nc.scalar.activation(ew2r[:, :], sc2p[:, :], AF.Exp)
            z2r = st.tile([128, B], F32, tag="z2r")
            nc.vector.tensor_reduce(
                z2r[:, :],
                ew2r[:, :].rearrange("p (b d) -> p b d", d=D),
                axis=mybir.AxisListType.X,
                op=OP.add,
            )
            rz2 = st.tile([128, B], F32, tag="rz2")
            nc.vector.reciprocal(rz2[:, :], z2r[:, :])
            tmp2 = st.tile([128, N], BF16, tag="tmp2")
            nc.vector.tensor_mul(tmp2[:, :], hs_bf[:, :], ew2r[:, :])
            ctx2r = st.tile([128, B], F32, tag="ctx2r")
            nc.vector.tensor_reduce(
                ctx2r[:, :],
                tmp2[:, :].rearrange("p (b d) -> p b d", d=D),
                axis=mybir.AxisListType.X,
                op=OP.add,
            )
            ctx2 = st.tile([128, B], F32, tag="ctx2")
            nc.vector.tensor_mul(ctx2[:, :], ctx2r[:, :], rz2[:, :])

            # ---------------- phase 6: per-stock head + global ----------
            y1ps = ps.tile([128, B], F32, tag="mm")
            nc.tensor.matmul(y1ps[:, :], x1w[:, :], ctx2[:, :], start=True, stop=True)
            y1 = st.tile([128, B], F32, tag="y1")
            nc.scalar.activation(y1[:, :], y1ps[:, :], AF.Relu, bias=x1b[:, 0:1])
            o2ps = ps.tile([64, B], F32, tag="mm")
            nc.tensor.matmul(o2ps[:, :], x2w[:, :], y1[:, :], start=True, stop=True)
            txt = st.tile([64, B], F32, tag="txt")
            nc.scalar.add(txt[:, :], o2ps[:, :], x2b[:, 0:1])
            nc.sync.dma_start(out=cc_in[:, :], in_=txt[:, :])
            nc.gpsimd.collective_compute(
                "AllGather",
                OP.bypass,
                replica_groups=[list(range(NCORES))],
                ins=[cc_in[:, :]],
                outs=[cc_out[:, :]],
            )
            # xs path
            y2ps = ps.tile([64, B], F32, tag="mm")
            nc.tensor.matmul(y2ps[:, :], h1w[:, :], sft[:, :], start=True, stop=True)
            y2 = st.tile([64, B], F32, tag="y2")
            nc.scalar.activation(y2[:, :], y2ps[:, :], AF.Relu, bias=h1b[:, 0:1])
            xsps = ps.tile([32, B], F32, tag="mm")
            nc.tensor.matmul(xsps[:, :], h2w[:, :], y2[:, :], start=True, stop=True)
            xst = st.tile([32, B], F32, tag="xst")
            nc.scalar.add(xst[:, :], xsps[:, :], h2b[:, 0:1])
            # final: out.T = tanh(hc_W.T @ [xs; text].T + hc_b)
            ga = st.tile([128, 4 * B], F32, tag="ga")
            for k in range(4):
                nc.sync.dma_start(
                    out=ga[:, k * B : (k + 1) * B], in_=cc_out[k * 128 : (k + 1) * 128, :]
                )
            fps = ps.tile([S, B], F32, tag="mm")
            nc.tensor.matmul(fps[:, :], hcw0[:, :], xst[:, :], start=True, stop=False)
            for k in range(4):
                nc.tensor.matmul(
                    fps[:, :], hcws[:, k * S : (k + 1) * S], ga[:, k * B : (k + 1) * B],
                    start=False, stop=(k == 3),
                )
            osb = st.tile([S, B], F32, tag="osb")
            nc.scalar.activation(osb[:, :], fps[:, :], AF.Tanh, bias=hcb[:, 0:1])
            nc.sync.dma_start(out=out_h[:, :], in_=osb[:, :])

    return nc


def make_in_maps(
    stock_feats, sentence_feat, time_feats, len_tweets,
    tl_Wall, tl_ball, tl_Uall, tl_bU, tl_Wd, tl_bd,
    a1_W1, a1_b1, a1_W2, a1_b2, a1_V, a1_bV,
    l2_Wih, l2_bih, l2_Whh, l2_bhh,
    a2_W1, a2_b1, a2_W2, a2_b2, a2_V, a2_bV,
    x1_W, x1_b, x2_W, x2_b,
    h1_W, h1_b, h2_W, h2_b, hc_W, hc_b,
):
    f32 = np.float32

    def permcols(w, perm):
        # w [..., 4*128] -> permuted gate blocks
        shp = w.shape
        wr = w.reshape(shp[:-1] + (4, 128))
        return wr[..., perm, :].reshape(shp)

    in_maps = []
    shared = {}
    shared["sft"] = np.ascontiguousarray(stock_feats.T).astype(f32)
    shared["h1w"] = np.asarray(h1_W, f32)
    shared["h1b"] = np.asarray(h1_b, f32).reshape(64, 1)
    shared["h2w"] = np.asarray(h2_W, f32)
    shared["h2b"] = np.asarray(h2_b, f32).reshape(32, 1)
    shared["hcw0"] = np.asarray(hc_W, f32)[:32]
    shared["hcws"] = np.ascontiguousarray(
        np.asarray(hc_W, f32)[32:].reshape(4, 128, S).transpose(1, 0, 2)
    ).reshape(128, 4 * S)
    shared["hcb"] = np.asarray(hc_b, f32).reshape(S, 1)
    shared["ident"] = np.eye(128, dtype=f32).astype(BF)

    for s in range(S):
        m = dict(shared)
        xs = np.asarray(sentence_feat[:, s], f32)          # [B, D, T, E]
        xbf = xs.astype(BF)                                # cast first (cheap)
        # [B, D, T, E] -> [E, T, B, D] -> [EB, 128, T*N]
        m["x"] = np.ascontiguousarray(xbf.transpose(3, 2, 0, 1)).reshape(EB, 128, TOK)
        tt = np.asarray(time_feats[:, s], f32)             # [B, D, T]
        m["tm1"] = (
            np.ascontiguousarray(tt.transpose(2, 0, 1)).reshape(1, TOK) - 1.0
        ).astype(f32)
        lens = np.asarray(len_tweets[:, s]).reshape(N)     # [N] int
        tgrid = np.arange(T)[:, None]
        m["maskbc"] = (tgrid < lens[None, :]).astype(f32).reshape(1, TOK).astype(BF)
        m["m1"] = (tgrid == (lens[None, :] - 1)).astype(f32).reshape(1, TOK).astype(BF)
        m["wd"] = np.asarray(tl_Wd[s], f32)
        m["bd"] = np.asarray(tl_bd[s], f32).reshape(H, 1)
        m["wall"] = permcols(np.asarray(tl_Wall[s], f32), PERM1).astype(BF)
        u = permcols(np.asarray(tl_Uall[s], f32), PERM1)   # [E, 512]
        m["uall"] = np.ascontiguousarray(
            u.reshape(EB, 128, H4).transpose(1, 0, 2)
        ).reshape(128, EB * H4).astype(BF)
        bgv = permcols(
            (np.asarray(tl_ball[s], f32) + np.asarray(tl_bU[s], f32))[None, :], PERM1
        )[0]
        m["bg"] = np.ascontiguousarray(bgv.reshape(4, 128).T).astype(f32)
        m["a1w1"] = np.asarray(a1_W1[s], f32).astype(BF)
        m["a1b1"] = np.asarray(a1_b1[s], f32).reshape(H, 1)
        m["a1w2"] = np.asarray(a1_W2[s], f32).astype(BF)
        m["a1b2"] = np.asarray(a1_b2[s], f32).reshape(H, 1)
        m["a1vr"] = np.tile(np.asarray(a1_V[s], f32).reshape(H, 1), (1, 128)).astype(BF)
        m["l2wih"] = permcols(np.asarray(l2_Wih[s], f32), PERM2).astype(BF)
        m["l2whh"] = permcols(np.asarray(l2_Whh[s], f32), PERM2).astype(BF)
        bl2v = permcols(
            (np.asarray(l2_bih[s], f32) + np.asarray(l2_bhh[s], f32))[None, :], PERM2
        )[0]
        m["bl2"] = np.ascontiguousarray(bl2v.reshape(4, 128).T).astype(f32)
        m["a2w1"] = np.asarray(a2_W1[s], f32).astype(BF)
        m["a2b1"] = np.asarray(a2_b1[s], f32).reshape(H, 1)
        m["a2w2"] = np.asarray(a2_W2[s], f32).astype(BF)
        m["a2b2"] = np.asarray(a2_b2[s], f32).reshape(H, 1)
        m["a2vr"] = np.tile(np.asarray(a2_V[s], f32).reshape(H, 1), (1, 128)).astype(BF)
        m["x1w"] = np.asarray(x1_W[s], f32)
        m["x1b"] = np.asarray(x1_b[s], f32).reshape(H, 1)
        m["x2w"] = np.asarray(x2_W[s], f32)
        m["x2b"] = np.asarray(x2_b[s], f32).reshape(64, 1)
        wbf = np.zeros((128, WBF_COLS), BF)
        off = 0
        for nm, rows, cols in W_BF:
            v = np.asarray(m.pop(nm))
            wbf[:rows, off : off + cols] = v
            off += cols
        m["wbf"] = wbf
        wf32 = np.zeros((128, WF32_COLS), f32)
        off = 0
        for nm, rows, cols in W_F32:
            v = np.asarray(m.pop(nm), f32).reshape(rows, cols)
            wf32[:rows, off : off + cols] = v
            off += cols
        m["wf32"] = wf32
        in_maps.append(m)
    return in_maps


_CACHED_NC = None
TRACE = False
LAST_EXEC_NS = None
LAST_RESULT = None


def kernel(**inputs) -> np.ndarray:
    global _CACHED_NC, LAST_EXEC_NS, LAST_RESULT
    from concourse.bass_utils import run_bass_kernel_spmd

    in_maps = make_in_maps(**inputs)
    if _CACHED_NC is None:
        nc = build_nc()
        nc.finalize()
        _CACHED_NC = nc
    res = run_bass_kernel_spmd(
        _CACHED_NC, in_maps, list(range(NCORES)), trace=TRACE
    )
    LAST_EXEC_NS = res.exec_time_ns
    LAST_RESULT = res
    out_t = res.results[0]["out"]          # [S, B]
    return np.ascontiguousarray(out_t.T).astype(np.float32)  # [B, S]


# revision 30
# speedup vs baseline: 1.1888x; 1.0461x over previous
"""Trainium2 Bass kernel for nn_Actor_73057393705109.

Architecture (per stock s, sharded one stock per NeuronCore, 8 cores):
  TimeLSTM over T=30 steps of B*D=160 sequences (E=768 -> H=128)
  -> masked attention over T -> day-LSTM over D=5 -> attention over D
  -> 2-layer MLP head per stock -> AllGather -> global linear head.

Device layout: "transposed" everywhere — feature dims on SBUF partitions,
sequence index n = b*D + d on the free dim. Matmul operands in bf16
(1 cyc/row on the PE), state and softmax math in fp32.
"""

import sys

if "/opt/trn_rl_repo" not in sys.path:
    sys.path.insert(0, "/opt/trn_rl_repo")

import ml_dtypes
import numpy as np

import concourse.bacc as bacc
import concourse.bass as bass
import concourse.mybir as mybir
from concourse import library_config
from concourse.tile import TileContext

F32 = mybir.dt.float32
BF16 = mybir.dt.bfloat16
AF = mybir.ActivationFunctionType
OP = mybir.AluOpType
BF = ml_dtypes.bfloat16

S, B, D, T, E, H = 8, 32, 5, 30, 768, 128
H4 = 4 * H
N = B * D            # 160 sequences per stock
TOK = T * N          # 4800 tokens, t-major: tok = t*N + n
EB = E // 128        # 6 e-blocks
TPC = 3              # t-steps per xU chunk
CH = TPC * N         # 480 tokens per chunk
NCH = T // TPC       # 10 chunks
NCORES = 8
import os
USE_GP_CADJ = os.environ.get("USE_GP_CADJ", "1") == "1"
USE_GP_ATTN = os.environ.get("USE_GP_ATTN", "1") == "1"


# packed weight layout: (name, rows, cols) concatenated along the free dim
W_BF = [("wall", 128, H4), ("uall", 128, EB * H4), ("ident", 128, 128),
        ("a1w1", 128, H), ("a1w2", 128, H), ("a1vr", 128, 128),
        ("l2wih", 128, H4), ("l2whh", 128, H4),
        ("a2w1", 128, H), ("a2w2", 128, H), ("a2vr", 128, 128)]
W_F32 = [("wd", 128, H), ("bd", 128, 1), ("bg", 128, 4),
         ("a1b1", 128, 1), ("a1b2", 128, 1), ("bl2", 128, 4),
         ("a2b1", 128, 1), ("a2b2", 128, 1),
         ("x1w", 128, H), ("x1b", 128, 1), ("x2w", 128, 64), ("x2b", 64, 1),
         ("sft", 17, B), ("h1w", 17, 64), ("h1b", 64, 1),
         ("h2w", 64, 32), ("h2b", 32, 1), ("hcw0", 32, S),
         ("hcb", 8, 1), ("hcws", 128, 4 * S)]
WBF_COLS = sum(c for _, _, c in W_BF)
WF32_COLS = sum(c for _, _, c in W_F32)

# gate permutation for the TimeLSTM: reference order (f, i, o, ct) -> (f, i, ct, o)
PERM1 = [0, 1, 3, 2]
# gate permutation for the day LSTM: reference order (i, f, g, o) -> (i, f, o, g)
PERM2 = [0, 1, 3, 2]


def _rep_ap(tile_ap, reps, inner):
    """AP reading [P, inner] tile as [P, reps, inner] with step-0 repeat."""
    return bass.AP(
        tensor=tile_ap.tensor,
        offset=tile_ap.offset,
        ap=[list(tile_ap.ap[0])] + [[0, reps], [1, inner]],
    )


def build_nc():
    nc = bacc.Bacc()

    def inp(name, shape, dtype=F32):
        return nc.declare_dram_parameter(name, shape, dtype, isOutput=False)

    x_h = inp("x", [EB, 128, TOK], BF16)
    tm1_h = inp("tm1", [1, TOK])
    mask_h = inp("maskbc", [1, TOK], BF16)
    m1_h = inp("m1", [1, TOK], BF16)
    wbf_h = inp("wbf", [128, WBF_COLS], BF16)
    wf32_h = inp("wf32", [128, WF32_COLS])

    out_h = nc.declare_dram_parameter("out", [S, B], F32, isOutput=True)

    # internal DRAM
    cc_in = nc.dram_tensor("cc_in", [64, B], F32)
    cc_out = nc.dram_tensor("cc_out", [S * 64, B], F32, addr_space="Shared")

    with TileContext(nc) as tc:
        with (
            tc.tile_pool(name="big", bufs=1) as big,
            tc.tile_pool(name="wpool", bufs=1) as wp,
            tc.tile_pool(name="state", bufs=1) as st,
            tc.tile_pool(name="xin", bufs=2) as xin,
            tc.tile_pool(name="work", bufs=2) as wk,
            tc.tile_pool(name="ps", bufs=2, space="PSUM") as ps,
        ):
            # ---------------- phase 0: weights (2 packed DMAs) ----------
            wbf_t = wp.tile([128, WBF_COLS], BF16, tag="wbf")
            nc.sync.dma_start(out=wbf_t[:, :], in_=wbf_h[:, :])
            wf32_t = wp.tile([128, WF32_COLS], F32, tag="wf32")
            nc.sync.dma_start(out=wf32_t[:, :], in_=wf32_h[:, :])

            def _mk_slices(table, tile):
                out, off = {}, 0
                for nm, rows, cols in table:
                    out[nm] = tile[0:rows, off : off + cols]
                    off += cols
                return out

            wsl = _mk_slices(W_BF, wbf_t)
            wsl.update(_mk_slices(W_F32, wf32_t))
            wall, uall, ident = wsl["wall"], wsl["uall"], wsl["ident"]
            a1w1, a1w2, a1vr = wsl["a1w1"], wsl["a1w2"], wsl["a1vr"]
            l2wih, l2whh = wsl["l2wih"], wsl["l2whh"]
            a2w1, a2w2, a2vr = wsl["a2w1"], wsl["a2w2"], wsl["a2vr"]
            wd, bd, bg, bl2 = wsl["wd"], wsl["bd"], wsl["bg"], wsl["bl2"]
            a1b1, a1b2, a2b1, a2b2 = wsl["a1b1"], wsl["a1b2"], wsl["a2b1"], wsl["a2b2"]
            x1w, x1b, x2w, x2b = wsl["x1w"], wsl["x1b"], wsl["x2w"], wsl["x2b"]
            sft, h1w, h1b = wsl["sft"], wsl["h1w"], wsl["h1b"]
            h2w, h2b, hcw0, hcb, hcws = wsl["h2w"], wsl["h2b"], wsl["hcw0"], wsl["hcb"], wsl["hcws"]

            maskbc = big.tile([128, TOK], BF16, tag="maskbc")
            tm1bc = big.tile([128, TOK], F32, tag="tm1bc")
            m1bc = big.tile([128, TOK], BF16, tag="m1bc")

            def tm1_load(ci):
                r = slice(ci * CH, (ci + 1) * CH)
                nc.scalar.dma_start(
                    out=tm1bc[:, r], in_=tm1_h[0:1, r].partition_broadcast(128)
                )

            # big persistent buffers
            xu = big.tile([128, 4 * TOK], BF16, tag="xu")
            obuf = big.tile([128, TOK], BF16, tag="obuf")

            # scan state
            h_bf = st.tile([128, N], BF16, tag="h_bf")
            c = st.tile([128, N], F32, tag="c")
            nc.vector.memzero(h_bf[:, :])
            nc.vector.memzero(c[:, :])

            # ------------- phases 1+2: xU production + scan -------------
            def xu_load(ci):
                t0 = ci * TPC
                # one consolidated chunk load: xT chunk [128, EB*CH]
                xT = xin.tile([128, EB * CH], BF16, tag="xTc")
                nc.sync.dma_start(
                    out=xT[:, :].rearrange("p (k c) -> p k c", k=EB),
                    in_=x_h[:, :, :].rearrange("k p c -> p k c")[
                        :, :, t0 * N : t0 * N + CH
                    ],
                )
                return xT

            def xu_j(ci, xT, j):
                # xu[j][chunk] = sum_k uall[k,j].T @ xT[k][chunk], bias folded
                t0 = ci * TPC
                pt = ps.tile([128, CH], F32, tag="xu")
                for k in range(EB):
                    nc.tensor.matmul(
                        pt[:, :],
                        uall[:, k * H4 + j * 128 : k * H4 + (j + 1) * 128],
                        xT[:, k * CH : k * CH + CH],
                        start=(k == 0),
                        stop=(k == EB - 1),
                    )
                dst = xu[:, j * TOK + t0 * N : j * TOK + t0 * N + CH]
                if j == 2:
                    nc.scalar.add(dst, pt[:, :], bg[:, j : j + 1])
                else:
                    nc.vector.tensor_scalar_add(dst, pt[:, :], bg[:, j : j + 1])

            def xu_chunk(ci):
                xT = xu_load(ci)
                for j in range(4):
                    xu_j(ci, xT, j)

            def scan_step(t):
                sl = slice(t * N, (t + 1) * N)
                # --- c-path first: depends only on c(t-1), overlaps the
                # previous step's h-path tail ---
                gB = ps.tile([128, 2 * N], F32, tag="gB")
                nc.tensor.matmul(
                    gB[:, N : 2 * N], wd[:, :], c[:, :],
                    start=True, stop=True, skip_group_check=True,
                )
                cs1 = st.tile([128, N], F32, tag="cs1")
                nc.scalar.activation(cs1[:, :], gB[:, N : 2 * N], AF.Tanh, bias=bd[:, 0:1])
                # c_adj = c + cs1 * tm1   (gpsimd, off the critical path)
                ca = st.tile([128, N], F32, tag="ca")
                nc.gpsimd.tensor_mul(ca[:, :], cs1[:, :], tm1bc[:, sl])
                nc.gpsimd.tensor_add(ca[:, :], ca[:, :], c[:, :])
                # --- h-path: gate matmuls need h_bf(t-1) ---
                gA = ps.tile([128, 3 * N], F32, tag="gA")
                for j in range(3):  # f, i, ct
                    r = slice(j * N, (j + 1) * N)
                    nc.tensor.matmul(
                        gA[:, r], wall[:, j * 128 : (j + 1) * 128], h_bf[:, :],
                        start=True, stop=False, skip_group_check=True,
                    )
                    nc.tensor.matmul(
                        gA[:, r], ident[:, :], xu[:, j * TOK + t * N : j * TOK + (t + 1) * N],
                        start=False, stop=True, skip_group_check=True,
                    )
                nc.tensor.matmul(
                    gB[:, 0:N], wall[:, 384:512], h_bf[:, :],
                    start=True, stop=False, skip_group_check=True,
                )
                nc.tensor.matmul(
                    gB[:, 0:N], ident[:, :], xu[:, 3 * TOK + t * N : 3 * TOK + (t + 1) * N],
                    start=False, stop=True, skip_group_check=True,
                )
                fic = st.tile([128, 3 * N], F32, tag="fic")
                nc.scalar.activation(fic[:, :], gA[:, :], AF.Sigmoid)
                nc.scalar.activation(obuf[:, sl], gB[:, 0:N], AF.Sigmoid)
                # c = f*c_adj + i*ct   (bv on gpsimd in parallel with av)
                av = st.tile([128, N], F32, tag="av")
                bv = st.tile([128, N], F32, tag="bv")
                nc.gpsimd.tensor_mul(bv[:, :], fic[:, N : 2 * N], fic[:, 2 * N : 3 * N])
                nc.vector.tensor_mul(av[:, :], fic[:, 0:N], ca[:, :])
                nc.vector.tensor_add(c[:, :], av[:, :], bv[:, :])
                tc2 = st.tile([128, N], BF16, tag="tc2")
                nc.scalar.activation(tc2[:, :], c[:, :], AF.Tanh)
                nc.vector.tensor_mul(h_bf[:, :], obuf[:, sl], tc2[:, :])

            # Fine-grained static interleave: the PE is in-order, so xU
            # matmuls for chunk ci+2 are woven between scan steps — while a
            # scan step waits on h_bf, the queued xU matmuls are NOT stuck
            # behind it across chunk boundaries (prefetch distance 2 keeps
            # every scan step's xu slice ready well in advance).
            nc.scalar.dma_start(
                out=m1bc[:, :], in_=m1_h[0:1, :].partition_broadcast(128)
            )
            s2buf = big.tile([128, TOK], F32, tag="s2buf")
            hn = st.tile([128, N], F32, tag="hn")
            nc.vector.memzero(hn[:, :])

            def attn_weave(ci):
                # runs after chunk ci's scan steps: hn partial + s2 chunk
                r = slice(ci * CH, (ci + 1) * CH)
                hm = wk.tile([128, CH], BF16, tag="hm")
                nc.vector.tensor_mul(hm[:, :], obuf[:, r], m1bc[:, r])
                hp = wk.tile([128, N], F32, tag="hp")
                nc.vector.tensor_reduce(
                    hp[:, :],
                    hm[:, :].rearrange("p (t n) -> p n t", t=TPC),
                    axis=mybir.AxisListType.X,
                    op=OP.add,
                )
                nc.vector.tensor_add(hn[:, :], hn[:, :], hp[:, :])
                sp = ps.tile([128, CH], F32, tag="mm")
                nc.tensor.matmul(sp[:, :], a1w2[:, :], obuf[:, r], start=True, stop=True)
                nc.scalar.copy(s2buf[:, r], sp[:, :])

            tm1_load(0)
            tm1_load(1)
            xu_chunk(0)
            xu_chunk(1)
            pre_xT = {}
            for ci in range(NCH):
                for dt_ in range(TPC):
                    scan_step(ci * TPC + dt_)
                    nxt = ci + 2
                    if nxt < NCH:
                        if dt_ == 0:
                            tm1_load(nxt)
                            pre_xT[nxt] = xu_load(nxt)
                            xu_j(nxt, pre_xT[nxt], 0)
                        elif dt_ == 1:
                            xu_j(nxt, pre_xT[nxt], 1)
                            xu_j(nxt, pre_xT[nxt], 2)
                        else:
                            xu_j(nxt, pre_xT.pop(nxt), 3)
                    if dt_ == TPC - 1:
                        attn_weave(ci)

            if os.environ.get("SKIP_TAIL", "0") == "1":
                osb0 = st.tile([S, B], F32, tag="osb0")
                nc.vector.tensor_copy(osb0[:, :], obuf[0:S, 0:B])
                nc.sync.dma_start(out=out_h[:, :], in_=osb0[:, :])
                return nc
            # attention-phase buffers (allocated after xT/xin released)
            nc.sync.dma_start(
                out=maskbc[:, :], in_=mask_h[0:1, :].partition_broadcast(128)
            )
            tmpbig = big.tile([128, TOK], BF16, tag="tmpbig")
            thout = big.tile([128, TOK], BF16, tag="thout")

            # ---------------- phase 3: attention over T -----------------
            hn_bf = st.tile([128, N], BF16, tag="hn_bf")
            nc.vector.tensor_copy(hn_bf[:, :], hn[:, :])
            s1ps = ps.tile([128, N], F32, tag="mm")
            nc.tensor.matmul(s1ps[:, :], a1w1[:, :], hn_bf[:, :], start=True, stop=True)
            s1 = st.tile([128, N], F32, tag="s1")
            nc.scalar.add(s1[:, :], s1ps[:, :], a1b1[:, 0:1])
            # th = tanh(s1 + W2.T @ obuf + b2), in 480-token chunks
            for ci in range(NCH):
                r = slice(ci * CH, (ci + 1) * CH)
                ti = wk.tile([128, CH], F32, tag="ti")
                nc.vector.tensor_add(
                    ti[:, :].rearrange("p (r n) -> p r n", r=TPC),
                    s2buf[:, r].rearrange("p (r n) -> p r n", r=TPC),
                    _rep_ap(s1[:, :], TPC, N),
                )
                nc.scalar.activation(thout[:, r], ti[:, :], AF.Tanh, bias=a1b2[:, 0:1])
            # scores replicated across partitions: lhsT = V tiled into all 128
            # columns, so out[p, tok] = sum_j V[j] th[j, tok] for every p.
            # The whole softmax then stays lane-local (no partition broadcast).
            neg30 = st.tile([128, 1], F32, tag="neg30")
            nc.vector.memset(neg30[:, :], -30.0)
            ewbc = big.tile([128, TOK], BF16, tag="m1bc")
            for ci in range(NCH):
                r = slice(ci * CH, (ci + 1) * CH)
                scp = ps.tile([128, CH], F32, tag="mm")
                nc.tensor.matmul(scp[:, :], a1vr[:, :], thout[:, r], start=True, stop=True)
                smc = wk.tile([128, CH], F32, tag="ti")
                nc.vector.scalar_tensor_tensor(
                    smc[:, :], scp[:, :], 30.0, maskbc[:, r], OP.add, OP.mult
                )
                nc.scalar.activation(ewbc[:, r], smc[:, :], AF.Exp, bias=neg30[:, 0:1])
            zr = st.tile([128, N], F32, tag="zr")
            nc.vector.tensor_reduce(
                zr[:, :],
                ewbc[:, :].rearrange("p (t n) -> p n t", t=T),
                axis=mybir.AxisListType.X,
                op=OP.add,
            )
            rz = st.tile([128, N], F32, tag="rz")
            nc.vector.reciprocal(rz[:, :], zr[:, :])
            nc.vector.tensor_mul(tmpbig[:, :], obuf[:, :], ewbc[:, :])
            ctxr = st.tile([128, N], F32, tag="ctxr")
            nc.vector.tensor_reduce(
                ctxr[:, :],
                tmpbig[:, :].rearrange("p (t n) -> p n t", t=T),
                axis=mybir.AxisListType.X,
                op=OP.add,
            )
            ctx_bf = st.tile([128, N], BF16, tag="ctx_bf")
            nc.vector.tensor_mul(ctx_bf[:, :], ctxr[:, :], rz[:, :])

            # ---------------- phase 4: day LSTM (D steps) ---------------
            hs_bf = st.tile([128, N], BF16, tag="hs_bf")
            h2st = st.tile([128, B], BF16, tag="h2st")
            c2st = st.tile([128, B], F32, tag="c2st")
            nc.vector.memzero(h2st[:, :])
            nc.vector.memzero(c2st[:, :])
            for d in range(D):
                xin_d = ctx_bf[:, :].rearrange("p (b d) -> p d b", d=D)[:, d, :]
                g2 = ps.tile([128, 4 * B], F32, tag="mm")
                for j in range(4):
                    r = slice(j * B, (j + 1) * B)
                    nc.tensor.matmul(
                        g2[:, r], l2wih[:, j * 128 : (j + 1) * 128], xin_d,
                        start=True, stop=False, skip_group_check=True,
                    )
                    nc.tensor.matmul(
                        g2[:, r], l2whh[:, j * 128 : (j + 1) * 128], h2st[:, :],
                        start=False, stop=True, skip_group_check=True,
                    )
                sg = st.tile([128, 3 * B], F32, tag="sg")
                for j in range(3):  # i, f, o
                    nc.scalar.activation(
                        sg[:, j * B : (j + 1) * B], g2[:, j * B : (j + 1) * B],
                        AF.Sigmoid, bias=bl2[:, j : j + 1],
                    )
                tg = st.tile([128, B], F32, tag="tg")
                nc.scalar.activation(tg[:, :], g2[:, 3 * B : 4 * B], AF.Tanh, bias=bl2[:, 3:4])
                a2v_ = st.tile([128, B], F32, tag="a2v_")
                b2v_ = st.tile([128, B], F32, tag="b2v_")
                nc.vector.tensor_mul(a2v_[:, :], sg[:, B : 2 * B], c2st[:, :])
                nc.vector.tensor_mul(b2v_[:, :], sg[:, 0:B], tg[:, :])
                nc.vector.tensor_add(c2st[:, :], a2v_[:, :], b2v_[:, :])
                tc2b = st.tile([128, B], BF16, tag="tc2b")
                nc.scalar.activation(tc2b[:, :], c2st[:, :], AF.Tanh)
                nc.vector.tensor_mul(h2st[:, :], sg[:, 2 * B : 3 * B], tc2b[:, :])
                nc.vector.tensor_copy(
                    hs_bf[:, :].rearrange("p (b d) -> p d b", d=D)[:, d, :], h2st[:, :]
                )

            # ---------------- phase 5: attention over D -----------------
            s1aps = ps.tile([128, B], F32, tag="mm")
            nc.tensor.matmul(s1aps[:, :], a2w1[:, :], h2st[:, :], start=True, stop=True)
            s1a = st.tile([128, B], F32, tag="s1a")
            nc.scalar.add(s1a[:, :], s1aps[:, :], a2b1[:, 0:1])
            s2aps = ps.tile([128, N], F32, tag="mm")
            nc.tensor.matmul(s2aps[:, :], a2w2[:, :], hs_bf[:, :], start=True, stop=True)
            t2i = st.tile([128, N], F32, tag="t2i")
            # hs layout is (b, d): s1a must repeat per-b along d -> use [b][d] view
            nc.vector.tensor_add(
                t2i[:, :].rearrange("p (b d) -> p b d", d=D),
                s2aps[:, :].rearrange("p (b d) -> p b d", d=D),
                bass.AP(
                    tensor=s1a.tensor,
                    offset=s1a[:, :].offset,
                    ap=[list(s1a[:, :].ap[0])] + [[1, B], [0, D]],
                ),
            )
            th2 = st.tile([128, N], BF16, tag="th2")
            nc.scalar.activation(th2[:, :], t2i[:, :], AF.Tanh, bias=a2b2[:, 0:1])
            # replicated scores again: out[p, (b,d)] = sum_j V2[j] th2[j, (b,d)]
            sc2p = ps.tile([128, N], F32, tag="mm")
            nc.tensor.matmul(sc2p[:, :], a2vr[:, :], th2[:, :], start=True, stop=True)
            ew2r = st.tile([128, N], BF16, tag="ew2r")
            nc.scalar.activation(ew2r[:, :], sc2p[:, :], AF.Exp)
            z2r = st.tile([128, B], F32, tag="z2r")
            nc.vector.tensor_reduce(
                z2r[:, :],
                ew2r[:, :].rearrange("p (b d) -> p b d", d=D),
                axis=mybir.AxisListType.X,
                op=OP.add,
            )
            rz2 = st.tile([128, B], F32, tag="rz2")
            nc.vector.reciprocal(rz2[:, :], z2r[:, :])
            tmp2 = st.tile([128, N], BF16, tag="tmp2")
            nc.vector.tensor_mul(tmp2[:, :], hs_bf[:, :], ew2r[:, :])
            ctx2r = st.tile([128, B], F32, tag="ctx2r")
            nc.vector.tensor_reduce(
                ctx2r[:, :],
                tmp2[:, :].rearrange("p (b d) -> p b d", d=D),
                axis=mybir.AxisListType.X,
                op=OP.add,
            )
            ctx2 = st.tile([128, B], F32, tag="ctx2")
            nc.vector.tensor_mul(ctx2[:, :], ctx2r[:, :], rz2[:, :])

            # ---------------- phase 6: per-stock head + global ----------
            y1ps = ps.tile([128, B], F32, tag="mm")
            nc.tensor.matmul(y1ps[:, :], x1w[:, :], ctx2[:, :], start=True, stop=True)
            y1 = st.tile([128, B], F32, tag="y1")
            nc.scalar.activation(y1[:, :], y1ps[:, :], AF.Relu, bias=x1b[:, 0:1])
            o2ps = ps.tile([64, B], F32, tag="mm")
            nc.tensor.matmul(o2ps[:, :], x2w[:, :], y1[:, :], start=True, stop=True)
            txt = st.tile([64, B], F32, tag="txt")
            nc.scalar.add(txt[:, :], o2ps[:, :], x2b[:, 0:1])
            nc.sync.dma_start(out=cc_in[:, :], in_=txt[:, :])
            nc.gpsimd.collective_compute(
                "AllGather",
                OP.bypass,
                replica_groups=[list(range(NCORES))],
                ins=[cc_in[:, :]],
                outs=[cc_out[:, :]],
            )
            # xs path
            y2ps = ps.tile([64, B], F32, tag="mm")
            nc.tensor.matmul(y2ps[:, :], h1w[:, :], sft[:, :], start=True, stop=True)
            y2 = st.tile([64, B], F32, tag="y2")
            nc.scalar.activation(y2[:, :], y2ps[:, :], AF.Relu, bias=h1b[:, 0:1])
            xsps = ps.tile([32, B], F32, tag="mm")
            nc.tensor.matmul(xsps[:, :], h2w[:, :], y2[:, :], start=True, stop=True)
            xst = st.tile([32, B], F32, tag="xst")
            nc.scalar.add(xst[:, :], xsps[:, :], h2b[:, 0:1])
            # final: out.T = tanh(hc_W.T @ [xs; text].T + hc_b)
            ga = st.tile([128, 4 * B], F32, tag="ga")
            for k in range(4):
                nc.sync.dma_start(
                    out=ga[:, k * B : (k + 1) * B], in_=cc_out[k * 128 : (k + 1) * 128, :]
                )
            fps = ps.tile([S, B], F32, tag="mm")
            nc.tensor.matmul(fps[:, :], hcw0[:, :], xst[:, :], start=True, stop=False)
            for k in range(4):
                nc.tensor.matmul(
                    fps[:, :], hcws[:, k * S : (k + 1) * S], ga[:, k * B : (k + 1) * B],
                    start=False, stop=(k == 3),
                )
            osb = st.tile([S, B], F32, tag="osb")
            nc.scalar.activation(osb[:, :], fps[:, :], AF.Tanh, bias=hcb[:, 0:1])
            nc.sync.dma_start(out=out_h[:, :], in_=osb[:, :])

    return nc


def make_in_maps(
    stock_feats, sentence_feat, time_feats, len_tweets,
    tl_Wall, tl_ball, tl_Uall, tl_bU, tl_Wd, tl_bd,
    a1_W1, a1_b1, a1_W2, a1_b2, a1_V, a1_bV,
    l2_Wih, l2_bih, l2_Whh, l2_bhh,
    a2_W1, a2_b1, a2_W2, a2_b2, a2_V, a2_bV,
    x1_W, x1_b, x2_W, x2_b,
    h1_W, h1_b, h2_W, h2_b, hc_W, hc_b,
):
    f32 = np.float32

    def permcols(w, perm):
        # w [..., 4*128] -> permuted gate blocks
        shp = w.shape
        wr = w.reshape(shp[:-1] + (4, 128))
        return wr[..., perm, :].reshape(shp)

    in_maps = []
    shared = {}
    shared["sft"] = np.ascontiguousarray(stock_feats.T).astype(f32)
    shared["h1w"] = np.asarray(h1_W, f32)
    shared["h1b"] = np.asarray(h1_b, f32).reshape(64, 1)
    shared["h2w"] = np.asarray(h2_W, f32)
    shared["h2b"] = np.asarray(h2_b, f32).reshape(32, 1)
    shared["hcw0"] = np.asarray(hc_W, f32)[:32]
    shared["hcws"] = np.ascontiguousarray(
        np.asarray(hc_W, f32)[32:].reshape(4, 128, S).transpose(1, 0, 2)
    ).reshape(128, 4 * S)
    shared["hcb"] = np.asarray(hc_b, f32).reshape(S, 1)
    shared["ident"] = np.eye(128, dtype=f32).astype(BF)

    for s in range(S):
        m = dict(shared)
        xs = np.asarray(sentence_feat[:, s], f32)          # [B, D, T, E]
        xbf = xs.astype(BF)                                # cast first (cheap)
        # [B, D, T, E] -> [E, T, B, D] -> [EB, 128, T*N]
        m["x"] = np.ascontiguousarray(xbf.transpose(3, 2, 0, 1)).reshape(EB, 128, TOK)
        tt = np.asarray(time_feats[:, s], f32)             # [B, D, T]
        m["tm1"] = (
            np.ascontiguousarray(tt.transpose(2, 0, 1)).reshape(1, TOK) - 1.0
        ).astype(f32)
        lens = np.asarray(len_tweets[:, s]).reshape(N)     # [N] int
        tgrid = np.arange(T)[:, None]
        m["maskbc"] = (tgrid < lens[None, :]).astype(f32).reshape(1, TOK).astype(BF)
        m["m1"] = (tgrid == (lens[None, :] - 1)).astype(f32).reshape(1, TOK).astype(BF)
        m["wd"] = np.asarray(tl_Wd[s], f32)
        m["bd"] = np.asarray(tl_bd[s], f32).reshape(H, 1)
        m["wall"] = permcols(np.asarray(tl_Wall[s], f32), PERM1).astype(BF)
        u = permcols(np.asarray(tl_Uall[s], f32), PERM1)   # [E, 512]
        m["uall"] = np.ascontiguousarray(
            u.reshape(EB, 128, H4).transpose(1, 0, 2)
        ).reshape(128, EB * H4).astype(BF)
        bgv = permcols(
            (np.asarray(tl_ball[s], f32) + np.asarray(tl_bU[s], f32))[None, :], PERM1
        )[0]
        m["bg"] = np.ascontiguousarray(bgv.reshape(4, 128).T).astype(f32)
        m["a1w1"] = np.asarray(a1_W1[s], f32).astype(BF)
        m["a1b1"] = np.asarray(a1_b1[s], f32).reshape(H, 1)
        m["a1w2"] = np.asarray(a1_W2[s], f32).astype(BF)
        m["a1b2"] = np.asarray(a1_b2[s], f32).reshape(H, 1)
        m["a1vr"] = np.tile(np.asarray(a1_V[s], f32).reshape(H, 1), (1, 128)).astype(BF)
        m["l2wih"] = permcols(np.asarray(l2_Wih[s], f32), PERM2).astype(BF)
        m["l2whh"] = permcols(np.asarray(l2_Whh[s], f32), PERM2).astype(BF)
        bl2v = permcols(
            (np.asarray(l2_bih[s], f32) + np.asarray(l2_bhh[s], f32))[None, :], PERM2
        )[0]
        m["bl2"] = np.ascontiguousarray(bl2v.reshape(4, 128).T).astype(f32)
        m["a2w1"] = np.asarray(a2_W1[s], f32).astype(BF)
        m["a2b1"] = np.asarray(a2_b1[s], f32).reshape(H, 1)
        m["a2w2"] = np.asarray(a2_W2[s], f32).astype(BF)
        m["a2b2"] = np.asarray(a2_b2[s], f32).reshape(H, 1)
        m["a2vr"] = np.tile(np.asarray(a2_V[s], f32).reshape(H, 1), (1, 128)).astype(BF)
        m["x1w"] = np.asarray(x1_W[s], f32)
        m["x1b"] = np.asarray(x1_b[s], f32).reshape(H, 1)
        m["x2w"] = np.asarray(x2_W[s], f32)
        m["x2b"] = np.asarray(x2_b[s], f32).reshape(64, 1)
        wbf = np.zeros((128, WBF_COLS), BF)
        off = 0
        for nm, rows, cols in W_BF:
            v = np.asarray(m.pop(nm))
            wbf[:rows, off : off + cols] = v
            off += cols
        m["wbf"] = wbf
        wf32 = np.zeros((128, WF32_COLS), f32)
        off = 0
        for nm, rows, cols in W_F32:
            v = np.asarray(m.pop(nm), f32).reshape(rows, cols)
            wf32[:rows, off : off + cols] = v
            off += cols
        m["wf32"] = wf32
        in_maps.append(m)
    return in_maps


_CACHED_NC = None
TRACE = False
LAST_EXEC_NS = None
LAST_RESULT = None


def kernel(**inputs) -> np.ndarray:
    global _CACHED_NC, LAST_EXEC_NS, LAST_RESULT
    from concourse.bass_utils import run_bass_kernel_spmd

    in_maps = make_in_maps(**inputs)
    if _CACHED_NC is None:
        nc = build_nc()
        nc.finalize()
        _CACHED_NC = nc
    res = run_bass_kernel_spmd(
        _CACHED_NC, in_maps, list(range(NCORES)), trace=TRACE
    )
    LAST_EXEC_NS = res.exec_time_ns
    LAST_RESULT = res
    out_t = res.results[0]["out"]          # [S, B]
    return np.ascontiguousarray(out_t.T).astype(np.float32)  # [B, S]


# revision 31
# speedup vs baseline: 1.2191x; 1.0255x over previous
"""Trainium2 Bass kernel for nn_Actor_73057393705109.

Architecture (per stock s, sharded one stock per NeuronCore, 8 cores):
  TimeLSTM over T=30 steps of B*D=160 sequences (E=768 -> H=128)
  -> masked attention over T -> day-LSTM over D=5 -> attention over D
  -> 2-layer MLP head per stock -> AllGather -> global linear head.

Device layout: "transposed" everywhere — feature dims on SBUF partitions,
sequence index n = b*D + d on the free dim. Matmul operands in bf16
(1 cyc/row on the PE), state and softmax math in fp32.
"""

import sys

if "/opt/trn_rl_repo" not in sys.path:
    sys.path.insert(0, "/opt/trn_rl_repo")

import ml_dtypes
import numpy as np

import concourse.bacc as bacc
import concourse.bass as bass
import concourse.mybir as mybir
from concourse import library_config
from concourse.tile import TileContext

F32 = mybir.dt.float32
BF16 = mybir.dt.bfloat16
AF = mybir.ActivationFunctionType
OP = mybir.AluOpType
BF = ml_dtypes.bfloat16

S, B, D, T, E, H = 8, 32, 5, 30, 768, 128
H4 = 4 * H
N = B * D            # 160 sequences per stock
TOK = T * N          # 4800 tokens, t-major: tok = t*N + n
EB = E // 128        # 6 e-blocks
TPC = 3              # t-steps per xU chunk
CH = TPC * N         # 480 tokens per chunk
NCH = T // TPC       # 10 chunks
NCORES = 8
import os
USE_GP_CADJ = os.environ.get("USE_GP_CADJ", "1") == "1"
USE_GP_ATTN = os.environ.get("USE_GP_ATTN", "1") == "1"


# packed weight layout: (name, rows, cols) concatenated along the free dim
W_BF = [("wall", 128, H4), ("uall", 128, EB * H4), ("ident", 128, 128),
        ("a1w1", 128, H), ("a1w2", 128, H), ("a1vr", 128, 128),
        ("l2wih", 128, H4), ("l2whh", 128, H4),
        ("a2w1", 128, H), ("a2w2", 128, H), ("a2vr", 128, 128)]
W_F32 = [("wd", 128, H), ("bd", 128, 1), ("bg", 128, 4),
         ("a1b1", 128, 1), ("a1b2", 128, 1), ("a1b12", 128, 1), ("bl2", 128, 4),
         ("a2b1", 128, 1), ("a2b2", 128, 1),
         ("x1w", 128, H), ("x1b", 128, 1), ("x2w", 128, 64), ("x2b", 64, 1),
         ("sft", 17, B), ("h1w", 17, 64), ("h1b", 64, 1),
         ("h2w", 64, 32), ("h2b", 32, 1), ("hcw0", 32, S),
         ("hcb", 8, 1), ("hcws", 128, 4 * S)]
WBF_COLS = sum(c for _, _, c in W_BF)
WF32_COLS = sum(c for _, _, c in W_F32)

# gate permutation for the TimeLSTM: reference order (f, i, o, ct) -> (f, i, ct, o)
PERM1 = [0, 1, 3, 2]
# gate permutation for the day LSTM: reference order (i, f, g, o) -> (i, f, o, g)
PERM2 = [0, 1, 3, 2]


def _rep_ap(tile_ap, reps, inner):
    """AP reading [P, inner] tile as [P, reps, inner] with step-0 repeat."""
    return bass.AP(
        tensor=tile_ap.tensor,
        offset=tile_ap.offset,
        ap=[list(tile_ap.ap[0])] + [[0, reps], [1, inner]],
    )


def build_nc():
    nc = bacc.Bacc()

    def inp(name, shape, dtype=F32):
        return nc.declare_dram_parameter(name, shape, dtype, isOutput=False)

    x_h = inp("x", [EB, 128, TOK], BF16)
    tm1_h = inp("tm1", [1, TOK])
    mask_h = inp("maskbc", [1, TOK], BF16)
    m1_h = inp("m1", [1, TOK], BF16)
    wbf_h = inp("wbf", [128, WBF_COLS], BF16)
    wf32_h = inp("wf32", [128, WF32_COLS])

    out_h = nc.declare_dram_parameter("out", [S, B], F32, isOutput=True)

    # internal DRAM
    cc_in = nc.dram_tensor("cc_in", [64, B], F32)
    cc_out = nc.dram_tensor("cc_out", [S * 64, B], F32, addr_space="Shared")

    with TileContext(nc) as tc:
        with (
            tc.tile_pool(name="big", bufs=1) as big,
            tc.tile_pool(name="wpool", bufs=1) as wp,
            tc.tile_pool(name="state", bufs=1) as st,
            tc.tile_pool(name="xin", bufs=2) as xin,
            tc.tile_pool(name="work", bufs=2) as wk,
            tc.tile_pool(name="ps", bufs=2, space="PSUM") as ps,
        ):
            # ---------------- phase 0: weights (2 packed DMAs) ----------
            wbf_t = wp.tile([128, WBF_COLS], BF16, tag="wbf")
            nc.sync.dma_start(out=wbf_t[:, :], in_=wbf_h[:, :])
            wf32_t = wp.tile([128, WF32_COLS], F32, tag="wf32")
            nc.sync.dma_start(out=wf32_t[:, :], in_=wf32_h[:, :])

            def _mk_slices(table, tile):
                out, off = {}, 0
                for nm, rows, cols in table:
                    out[nm] = tile[0:rows, off : off + cols]
                    off += cols
                return out

            wsl = _mk_slices(W_BF, wbf_t)
            wsl.update(_mk_slices(W_F32, wf32_t))
            wall, uall, ident = wsl["wall"], wsl["uall"], wsl["ident"]
            a1w1, a1w2, a1vr = wsl["a1w1"], wsl["a1w2"], wsl["a1vr"]
            l2wih, l2whh = wsl["l2wih"], wsl["l2whh"]
            a2w1, a2w2, a2vr = wsl["a2w1"], wsl["a2w2"], wsl["a2vr"]
            wd, bd, bg, bl2 = wsl["wd"], wsl["bd"], wsl["bg"], wsl["bl2"]
            a1b1, a1b2, a2b1, a2b2 = wsl["a1b1"], wsl["a1b2"], wsl["a2b1"], wsl["a2b2"]
            a1b12 = wsl["a1b12"]
            x1w, x1b, x2w, x2b = wsl["x1w"], wsl["x1b"], wsl["x2w"], wsl["x2b"]
            sft, h1w, h1b = wsl["sft"], wsl["h1w"], wsl["h1b"]
            h2w, h2b, hcw0, hcb, hcws = wsl["h2w"], wsl["h2b"], wsl["hcw0"], wsl["hcb"], wsl["hcws"]

            maskbc = big.tile([128, TOK], BF16, tag="maskbc")
            tm1bc = big.tile([128, TOK], F32, tag="tm1bc")
            m1bc = big.tile([128, TOK], BF16, tag="m1bc")

            def tm1_load(ci):
                r = slice(ci * CH, (ci + 1) * CH)
                nc.scalar.dma_start(
                    out=tm1bc[:, r], in_=tm1_h[0:1, r].partition_broadcast(128)
                )

            # big persistent buffers
            xu = big.tile([128, 4 * TOK], BF16, tag="xu")
            obuf = big.tile([128, TOK], BF16, tag="obuf")

            # scan state
            h_bf = st.tile([128, N], BF16, tag="h_bf")
            c = st.tile([128, N], F32, tag="c")
            nc.vector.memzero(h_bf[:, :])
            nc.vector.memzero(c[:, :])

            # ------------- phases 1+2: xU production + scan -------------
            def xu_load(ci):
                t0 = ci * TPC
                # one consolidated chunk load: xT chunk [128, EB*CH]
                xT = xin.tile([128, EB * CH], BF16, tag="xTc")
                nc.sync.dma_start(
                    out=xT[:, :].rearrange("p (k c) -> p k c", k=EB),
                    in_=x_h[:, :, :].rearrange("k p c -> p k c")[
                        :, :, t0 * N : t0 * N + CH
                    ],
                )
                return xT

            def xu_j(ci, xT, j):
                # xu[j][chunk] = sum_k uall[k,j].T @ xT[k][chunk], bias folded
                t0 = ci * TPC
                pt = ps.tile([128, CH], F32, tag="xu")
                for k in range(EB):
                    nc.tensor.matmul(
                        pt[:, :],
                        uall[:, k * H4 + j * 128 : k * H4 + (j + 1) * 128],
                        xT[:, k * CH : k * CH + CH],
                        start=(k == 0),
                        stop=(k == EB - 1),
                    )
                dst = xu[:, j * TOK + t0 * N : j * TOK + t0 * N + CH]
                if j == 2:
                    nc.scalar.add(dst, pt[:, :], bg[:, j : j + 1])
                else:
                    nc.vector.tensor_scalar_add(dst, pt[:, :], bg[:, j : j + 1])

            def xu_chunk(ci):
                xT = xu_load(ci)
                for j in range(4):
                    xu_j(ci, xT, j)

            def scan_step(t):
                sl = slice(t * N, (t + 1) * N)
                # --- c-path first: depends only on c(t-1), overlaps the
                # previous step's h-path tail ---
                gB = ps.tile([128, 2 * N], F32, tag="gB")
                nc.tensor.matmul(
                    gB[:, N : 2 * N], wd[:, :], c[:, :],
                    start=True, stop=True, skip_group_check=True,
                )
                cs1 = st.tile([128, N], F32, tag="cs1")
                nc.scalar.activation(cs1[:, :], gB[:, N : 2 * N], AF.Tanh, bias=bd[:, 0:1])
                # c_adj = c + cs1 * tm1   (gpsimd, off the critical path)
                ca = st.tile([128, N], F32, tag="ca")
                nc.gpsimd.tensor_mul(ca[:, :], cs1[:, :], tm1bc[:, sl])
                nc.gpsimd.tensor_add(ca[:, :], ca[:, :], c[:, :])
                # --- h-path: gate matmuls need h_bf(t-1) ---
                gA = ps.tile([128, 3 * N], F32, tag="gA")
                for j in range(3):  # f, i, ct
                    r = slice(j * N, (j + 1) * N)
                    nc.tensor.matmul(
                        gA[:, r], wall[:, j * 128 : (j + 1) * 128], h_bf[:, :],
                        start=True, stop=False, skip_group_check=True,
                    )
                    nc.tensor.matmul(
                        gA[:, r], ident[:, :], xu[:, j * TOK + t * N : j * TOK + (t + 1) * N],
                        start=False, stop=True, skip_group_check=True,
                    )
                nc.tensor.matmul(
                    gB[:, 0:N], wall[:, 384:512], h_bf[:, :],
                    start=True, stop=False, skip_group_check=True,
                )
                nc.tensor.matmul(
                    gB[:, 0:N], ident[:, :], xu[:, 3 * TOK + t * N : 3 * TOK + (t + 1) * N],
                    start=False, stop=True, skip_group_check=True,
                )
                fic = st.tile([128, 3 * N], F32, tag="fic")
                nc.scalar.activation(fic[:, :], gA[:, :], AF.Sigmoid)
                nc.scalar.activation(obuf[:, sl], gB[:, 0:N], AF.Sigmoid)
                # c = f*c_adj + i*ct   (bv on gpsimd in parallel with av)
                av = st.tile([128, N], F32, tag="av")
                bv = st.tile([128, N], F32, tag="bv")
                nc.gpsimd.tensor_mul(bv[:, :], fic[:, N : 2 * N], fic[:, 2 * N : 3 * N])
                nc.vector.tensor_mul(av[:, :], fic[:, 0:N], ca[:, :])
                nc.vector.tensor_add(c[:, :], av[:, :], bv[:, :])
                tc2 = st.tile([128, N], BF16, tag="tc2")
                nc.scalar.activation(tc2[:, :], c[:, :], AF.Tanh)
                nc.vector.tensor_mul(h_bf[:, :], obuf[:, sl], tc2[:, :])

            # Fine-grained static interleave: the PE is in-order, so xU
            # matmuls for chunk ci+2 are woven between scan steps — while a
            # scan step waits on h_bf, the queued xU matmuls are NOT stuck
            # behind it across chunk boundaries (prefetch distance 2 keeps
            # every scan step's xu slice ready well in advance).
            nc.scalar.dma_start(
                out=m1bc[:, :], in_=m1_h[0:1, :].partition_broadcast(128)
            )
            hn = st.tile([128, N], F32, tag="hn")
            nc.vector.memzero(hn[:, :])

            def attn_weave(ci):
                # runs after chunk ci's scan steps: hn masked partial
                r = slice(ci * CH, (ci + 1) * CH)
                hm = wk.tile([128, CH], BF16, tag="hm")
                nc.vector.tensor_mul(hm[:, :], obuf[:, r], m1bc[:, r])
                hp = wk.tile([128, N], F32, tag="hp")
                nc.vector.tensor_reduce(
                    hp[:, :],
                    hm[:, :].rearrange("p (t n) -> p n t", t=TPC),
                    axis=mybir.AxisListType.X,
                    op=OP.add,
                )
                nc.vector.tensor_add(hn[:, :], hn[:, :], hp[:, :])

            tm1_load(0)
            tm1_load(1)
            xu_chunk(0)
            xu_chunk(1)
            pre_xT = {}
            for ci in range(NCH):
                for dt_ in range(TPC):
                    scan_step(ci * TPC + dt_)
                    nxt = ci + 2
                    if nxt < NCH:
                        if dt_ == 0:
                            tm1_load(nxt)
                            pre_xT[nxt] = xu_load(nxt)
                            xu_j(nxt, pre_xT[nxt], 0)
                        elif dt_ == 1:
                            xu_j(nxt, pre_xT[nxt], 1)
                            xu_j(nxt, pre_xT[nxt], 2)
                        else:
                            xu_j(nxt, pre_xT.pop(nxt), 3)
                    if dt_ == TPC - 1:
                        attn_weave(ci)

            if os.environ.get("SKIP_TAIL", "0") == "1":
                osb0 = st.tile([S, B], F32, tag="osb0")
                nc.vector.tensor_copy(osb0[:, :], obuf[0:S, 0:B])
                nc.sync.dma_start(out=out_h[:, :], in_=osb0[:, :])
                return nc
            # attention-phase buffers (allocated after xT/xin released)
            nc.sync.dma_start(
                out=maskbc[:, :], in_=mask_h[0:1, :].partition_broadcast(128)
            )
            tmpbig = big.tile([128, TOK], BF16, tag="tmpbig")
            thout = big.tile([128, TOK], BF16, tag="thout")

            # ---------------- phase 3: attention over T -----------------
            hn_bf = st.tile([128, N], BF16, tag="hn_bf")
            nc.vector.tensor_copy(hn_bf[:, :], hn[:, :])
            # th = tanh(W2.T @ obuf + W1.T @ hn (repeated) + b1 + b2) per chunk;
            # the s1 broadcast rides the PE via a step-0-repeat rhs AP.
            for ci in range(NCH):
                r = slice(ci * CH, (ci + 1) * CH)
                sp = ps.tile([128, CH], F32, tag="mm")
                nc.tensor.matmul(sp[:, :], a1w2[:, :], obuf[:, r], start=True, stop=False, skip_group_check=True)
                nc.tensor.matmul(
                    sp[:, :].rearrange("p (r n) -> p r n", r=TPC),
                    a1w1[:, :], _rep_ap(hn_bf[:, :], TPC, N),
                    start=False, stop=True, skip_group_check=True,
                )
                nc.scalar.activation(thout[:, r], sp[:, :], AF.Tanh, bias=a1b12[:, 0:1])
            # scores replicated across partitions: lhsT = V tiled into all 128
            # columns, so out[p, tok] = sum_j V[j] th[j, tok] for every p.
            # The whole softmax then stays lane-local (no partition broadcast).
            neg30 = st.tile([128, 1], F32, tag="neg30")
            nc.vector.memset(neg30[:, :], -30.0)
            ewbc = big.tile([128, TOK], BF16, tag="m1bc")
            for ci in range(NCH):
                r = slice(ci * CH, (ci + 1) * CH)
                scp = ps.tile([128, CH], F32, tag="mm")
                nc.tensor.matmul(scp[:, :], a1vr[:, :], thout[:, r], start=True, stop=True)
                smc = wk.tile([128, CH], F32, tag="ti")
                nc.vector.scalar_tensor_tensor(
                    smc[:, :], scp[:, :], 30.0, maskbc[:, r], OP.add, OP.mult
                )
                nc.scalar.activation(ewbc[:, r], smc[:, :], AF.Exp, bias=neg30[:, 0:1])
            zr = st.tile([128, N], F32, tag="zr")
            nc.vector.tensor_reduce(
                zr[:, :],
                ewbc[:, :].rearrange("p (t n) -> p n t", t=T),
                axis=mybir.AxisListType.X,
                op=OP.add,
            )
            rz = st.tile([128, N], F32, tag="rz")
            nc.vector.reciprocal(rz[:, :], zr[:, :])
            nc.vector.tensor_mul(tmpbig[:, :], obuf[:, :], ewbc[:, :])
            ctxr = st.tile([128, N], F32, tag="ctxr")
            nc.vector.tensor_reduce(
                ctxr[:, :],
                tmpbig[:, :].rearrange("p (t n) -> p n t", t=T),
                axis=mybir.AxisListType.X,
                op=OP.add,
            )
            ctx_bf = st.tile([128, N], BF16, tag="ctx_bf")
            nc.vector.tensor_mul(ctx_bf[:, :], ctxr[:, :], rz[:, :])

            # ---------------- phase 4: day LSTM (D steps) ---------------
            hs_bf = st.tile([128, N], BF16, tag="hs_bf")
            h2st = st.tile([128, B], BF16, tag="h2st")
            c2st = st.tile([128, B], F32, tag="c2st")
            nc.vector.memzero(h2st[:, :])
            nc.vector.memzero(c2st[:, :])
            for d in range(D):
                xin_d = ctx_bf[:, :].rearrange("p (b d) -> p d b", d=D)[:, d, :]
                g2 = ps.tile([128, 4 * B], F32, tag="mm")
                for j in range(4):
                    r = slice(j * B, (j + 1) * B)
                    nc.tensor.matmul(
                        g2[:, r], l2wih[:, j * 128 : (j + 1) * 128], xin_d,
                        start=True, stop=False, skip_group_check=True,
                    )
                    nc.tensor.matmul(
                        g2[:, r], l2whh[:, j * 128 : (j + 1) * 128], h2st[:, :],
                        start=False, stop=True, skip_group_check=True,
                    )
                sg = st.tile([128, 3 * B], F32, tag="sg")
                for j in range(3):  # i, f, o
                    nc.scalar.activation(
                        sg[:, j * B : (j + 1) * B], g2[:, j * B : (j + 1) * B],
                        AF.Sigmoid, bias=bl2[:, j : j + 1],
                    )
                tg = st.tile([128, B], F32, tag="tg")
                nc.scalar.activation(tg[:, :], g2[:, 3 * B : 4 * B], AF.Tanh, bias=bl2[:, 3:4])
                a2v_ = st.tile([128, B], F32, tag="a2v_")
                b2v_ = st.tile([128, B], F32, tag="b2v_")
                nc.vector.tensor_mul(a2v_[:, :], sg[:, B : 2 * B], c2st[:, :])
                nc.vector.tensor_mul(b2v_[:, :], sg[:, 0:B], tg[:, :])
                nc.vector.tensor_add(c2st[:, :], a2v_[:, :], b2v_[:, :])
                tc2b = st.tile([128, B], BF16, tag="tc2b")
                nc.scalar.activation(tc2b[:, :], c2st[:, :], AF.Tanh)
                nc.vector.tensor_mul(h2st[:, :], sg[:, 2 * B : 3 * B], tc2b[:, :])
                nc.vector.tensor_copy(
                    hs_bf[:, :].rearrange("p (b d) -> p d b", d=D)[:, d, :], h2st[:, :]
                )

            # ---------------- phase 5: attention over D -----------------
            s1aps = ps.tile([128, B], F32, tag="mm")
            nc.tensor.matmul(s1aps[:, :], a2w1[:, :], h2st[:, :], start=True, stop=True)
            s1a = st.tile([128, B], F32, tag="s1a")
            nc.scalar.add(s1a[:, :], s1aps[:, :], a2b1[:, 0:1])
            s2aps = ps.tile([128, N], F32, tag="mm")
            nc.tensor.matmul(s2aps[:, :], a2w2[:, :], hs_bf[:, :], start=True, stop=True)
            t2i = st.tile([128, N], F32, tag="t2i")
            # hs layout is (b, d): s1a must repeat per-b along d -> use [b][d] view
            nc.vector.tensor_add(
                t2i[:, :].rearrange("p (b d) -> p b d", d=D),
                s2aps[:, :].rearrange("p (b d) -> p b d", d=D),
                bass.AP(
                    tensor=s1a.tensor,
                    offset=s1a[:, :].offset,
                    ap=[list(s1a[:, :].ap[0])] + [[1, B], [0, D]],
                ),
            )
            th2 = st.tile([128, N], BF16, tag="th2")
            nc.scalar.activation(th2[:, :], t2i[:, :], AF.Tanh, bias=a2b2[:, 0:1])
            # replicated scores again: out[p, (b,d)] = sum_j V2[j] th2[j, (b,d)]
            sc2p = ps.tile([128, N], F32, tag="mm")
            nc.tensor.matmul(sc2p[:, :], a2vr[:, :], th2[:, :], start=True, stop=True)
            ew2r = st.tile([128, N], BF16, tag="ew2r")
            nc.scalar.activation(ew2r[:, :], sc2p[:, :], AF.Exp)
            z2r = st.tile([128, B], F32, tag="z2r")
            nc.vector.tensor_reduce(
                z2r[:, :],
                ew2r[:, :].rearrange("p (b d) -> p b d", d=D),
                axis=mybir.AxisListType.X,
                op=OP.add,
            )
            rz2 = st.tile([128, B], F32, tag="rz2")
            nc.vector.reciprocal(rz2[:, :], z2r[:, :])
            tmp2 = st.tile([128, N], BF16, tag="tmp2")
            nc.vector.tensor_mul(tmp2[:, :], hs_bf[:, :], ew2r[:, :])
            ctx2r = st.tile([128, B], F32, tag="ctx2r")
            nc.vector.tensor_reduce(
                ctx2r[:, :],
                tmp2[:, :].rearrange("p (b d) -> p b d", d=D),
                axis=mybir.AxisListType.X,
                op=OP.add,
            )
            ctx2 = st.tile([128, B], F32, tag="ctx2")
            nc.vector.tensor_mul(ctx2[:, :], ctx2r[:, :], rz2[:, :])

            # ---------------- phase 6: per-stock head + global ----------
            y1ps = ps.tile([128, B], F32, tag="mm")
            nc.tensor.matmul(y1ps[:, :], x1w[:, :], ctx2[:, :], start=True, stop=True)
            y1 = st.tile([128, B], F32, tag="y1")
            nc.scalar.activation(y1[:, :], y1ps[:, :], AF.Relu, bias=x1b[:, 0:1])
            o2ps = ps.tile([64, B], F32, tag="mm")
            nc.tensor.matmul(o2ps[:, :], x2w[:, :], y1[:, :], start=True, stop=True)
            txt = st.tile([64, B], F32, tag="txt")
            nc.scalar.add(txt[:, :], o2ps[:, :], x2b[:, 0:1])
            nc.sync.dma_start(out=cc_in[:, :], in_=txt[:, :])
            nc.gpsimd.collective_compute(
                "AllGather",
                OP.bypass,
                replica_groups=[list(range(NCORES))],
                ins=[cc_in[:, :]],
                outs=[cc_out[:, :]],
            )
            # xs path
            y2ps = ps.tile([64, B], F32, tag="mm")
            nc.tensor.matmul(y2ps[:, :], h1w[:, :], sft[:, :], start=True, stop=True)
            y2 = st.tile([64, B], F32, tag="y2")
            nc.scalar.activation(y2[:, :], y2ps[:, :], AF.Relu, bias=h1b[:, 0:1])
            xsps = ps.tile([32, B], F32, tag="mm")
            nc.tensor.matmul(xsps[:, :], h2w[:, :], y2[:, :], start=True, stop=True)
            xst = st.tile([32, B], F32, tag="xst")
            nc.scalar.add(xst[:, :], xsps[:, :], h2b[:, 0:1])
            # final: out.T = tanh(hc_W.T @ [xs; text].T + hc_b)
            ga = st.tile([128, 4 * B], F32, tag="ga")
            for k in range(4):
                nc.sync.dma_start(
                    out=ga[:, k * B : (k + 1) * B], in_=cc_out[k * 128 : (k + 1) * 128, :]
                )
            fps = ps.tile([S, B], F32, tag="mm")
            nc.tensor.matmul(fps[:, :], hcw0[:, :], xst[:, :], start=True, stop=False)
            for k in range(4):
                nc.tensor.matmul(
                    fps[:, :], hcws[:, k * S : (k + 1) * S], ga[:, k * B : (k + 1) * B],
                    start=False, stop=(k == 3),
                )
            osb = st.tile([S, B], F32, tag="osb")
            nc.scalar.activation(osb[:, :], fps[:, :], AF.Tanh, bias=hcb[:, 0:1])
            nc.sync.dma_start(out=out_h[:, :], in_=osb[:, :])

    return nc


def make_in_maps(
    stock_feats, sentence_feat, time_feats, len_tweets,
    tl_Wall, tl_ball, tl_Uall, tl_bU, tl_Wd, tl_bd,
    a1_W1, a1_b1, a1_W2, a1_b2, a1_V, a1_bV,
    l2_Wih, l2_bih, l2_Whh, l2_bhh,
    a2_W1, a2_b1, a2_W2, a2_b2, a2_V, a2_bV,
    x1_W, x1_b, x2_W, x2_b,
    h1_W, h1_b, h2_W, h2_b, hc_W, hc_b,
):
    f32 = np.float32

    def permcols(w, perm):
        # w [..., 4*128] -> permuted gate blocks
        shp = w.shape
        wr = w.reshape(shp[:-1] + (4, 128))
        return wr[..., perm, :].reshape(shp)

    in_maps = []
    shared = {}
    shared["sft"] = np.ascontiguousarray(stock_feats.T).astype(f32)
    shared["h1w"] = np.asarray(h1_W, f32)
    shared["h1b"] = np.asarray(h1_b, f32).reshape(64, 1)
    shared["h2w"] = np.asarray(h2_W, f32)
    shared["h2b"] = np.asarray(h2_b, f32).reshape(32, 1)
    shared["hcw0"] = np.asarray(hc_W, f32)[:32]
    shared["hcws"] = np.ascontiguousarray(
        np.asarray(hc_W, f32)[32:].reshape(4, 128, S).transpose(1, 0, 2)
    ).reshape(128, 4 * S)
    shared["hcb"] = np.asarray(hc_b, f32).reshape(S, 1)
    shared["ident"] = np.eye(128, dtype=f32).astype(BF)

    for s in range(S):
        m = dict(shared)
        xs = np.asarray(sentence_feat[:, s], f32)          # [B, D, T, E]
        xbf = xs.astype(BF)                                # cast first (cheap)
        # [B, D, T, E] -> [E, T, B, D] -> [EB, 128, T*N]
        m["x"] = np.ascontiguousarray(xbf.transpose(3, 2, 0, 1)).reshape(EB, 128, TOK)
        tt = np.asarray(time_feats[:, s], f32)             # [B, D, T]
        m["tm1"] = (
            np.ascontiguousarray(tt.transpose(2, 0, 1)).reshape(1, TOK) - 1.0
        ).astype(f32)
        lens = np.asarray(len_tweets[:, s]).reshape(N)     # [N] int
        tgrid = np.arange(T)[:, None]
        m["maskbc"] = (tgrid < lens[None, :]).astype(f32).reshape(1, TOK).astype(BF)
        m["m1"] = (tgrid == (lens[None, :] - 1)).astype(f32).reshape(1, TOK).astype(BF)
        m["wd"] = np.asarray(tl_Wd[s], f32)
        m["bd"] = np.asarray(tl_bd[s], f32).reshape(H, 1)
        m["wall"] = permcols(np.asarray(tl_Wall[s], f32), PERM1).astype(BF)
        u = permcols(np.asarray(tl_Uall[s], f32), PERM1)   # [E, 512]
        m["uall"] = np.ascontiguousarray(
            u.reshape(EB, 128, H4).transpose(1, 0, 2)
        ).reshape(128, EB * H4).astype(BF)
        bgv = permcols(
            (np.asarray(tl_ball[s], f32) + np.asarray(tl_bU[s], f32))[None, :], PERM1
        )[0]
        m["bg"] = np.ascontiguousarray(bgv.reshape(4, 128).T).astype(f32)
        m["a1w1"] = np.asarray(a1_W1[s], f32).astype(BF)
        m["a1b1"] = np.asarray(a1_b1[s], f32).reshape(H, 1)
        m["a1w2"] = np.asarray(a1_W2[s], f32).astype(BF)
        m["a1b2"] = np.asarray(a1_b2[s], f32).reshape(H, 1)
        m["a1b12"] = (np.asarray(a1_b1[s], f32) + np.asarray(a1_b2[s], f32)).reshape(H, 1)
        m["a1vr"] = np.tile(np.asarray(a1_V[s], f32).reshape(H, 1), (1, 128)).astype(BF)
        m["l2wih"] = permcols(np.asarray(l2_Wih[s], f32), PERM2).astype(BF)
        m["l2whh"] = permcols(np.asarray(l2_Whh[s], f32), PERM2).astype(BF)
        bl2v = permcols(
            (np.asarray(l2_bih[s], f32) + np.asarray(l2_bhh[s], f32))[None, :], PERM2
        )[0]
        m["bl2"] = np.ascontiguousarray(bl2v.reshape(4, 128).T).astype(f32)
        m["a2w1"] = np.asarray(a2_W1[s], f32).astype(BF)
        m["a2b1"] = np.asarray(a2_b1[s], f32).reshape(H, 1)
        m["a2w2"] = np.asarray(a2_W2[s], f32).astype(BF)
        m["a2b2"] = np.asarray(a2_b2[s], f32).reshape(H, 1)
        m["a2vr"] = np.tile(np.asarray(a2_V[s], f32).reshape(H, 1), (1, 128)).astype(BF)
        m["x1w"] = np.asarray(x1_W[s], f32)
        m["x1b"] = np.asarray(x1_b[s], f32).reshape(H, 1)
        m["x2w"] = np.asarray(x2_W[s], f32)
        m["x2b"] = np.asarray(x2_b[s], f32).reshape(64, 1)
        wbf = np.zeros((128, WBF_COLS), BF)
        off = 0
        for nm, rows, cols in W_BF:
            v = np.asarray(m.pop(nm))
            wbf[:rows, off : off + cols] = v
            off += cols
        m["wbf"] = wbf
        wf32 = np.zeros((128, WF32_COLS), f32)
        off = 0
        for nm, rows, cols in W_F32:
            v = np.asarray(m.pop(nm), f32).reshape(rows, cols)
            wf32[:rows, off : off + cols] = v
            off += cols
        m["wf32"] = wf32
        in_maps.append(m)
    return in_maps


_CACHED_NC = None
TRACE = False
LAST_EXEC_NS = None
LAST_RESULT = None


def kernel(**inputs) -> np.ndarray:
    global _CACHED_NC, LAST_EXEC_NS, LAST_RESULT
    from concourse.bass_utils import run_bass_kernel_spmd

    in_maps = make_in_maps(**inputs)
    if _CACHED_NC is None:
        nc = build_nc()
        nc.finalize()
        _CACHED_NC = nc
    res = run_bass_kernel_spmd(
        _CACHED_NC, in_maps, list(range(NCORES)), trace=TRACE
    )
    LAST_EXEC_NS = res.exec_time_ns
    LAST_RESULT = res
    out_t = res.results[0]["out"]          # [S, B]
    return np.ascontiguousarray(out_t.T).astype(np.float32)  # [B, S]


# revision 32
# speedup vs baseline: 1.2265x; 1.0061x over previous
"""Trainium2 Bass kernel for nn_Actor_73057393705109.

Architecture (per stock s, sharded one stock per NeuronCore, 8 cores):
  TimeLSTM over T=30 steps of B*D=160 sequences (E=768 -> H=128)
  -> masked attention over T -> day-LSTM over D=5 -> attention over D
  -> 2-layer MLP head per stock -> AllGather -> global linear head.

Device layout: "transposed" everywhere — feature dims on SBUF partitions,
sequence index n = b*D + d on the free dim. Matmul operands in bf16
(1 cyc/row on the PE), state and softmax math in fp32.
"""

import sys

if "/opt/trn_rl_repo" not in sys.path:
    sys.path.insert(0, "/opt/trn_rl_repo")

import ml_dtypes
import numpy as np

import concourse.bacc as bacc
import concourse.bass as bass
import concourse.mybir as mybir
from concourse import library_config
from concourse.tile import TileContext

F32 = mybir.dt.float32
BF16 = mybir.dt.bfloat16
AF = mybir.ActivationFunctionType
OP = mybir.AluOpType
BF = ml_dtypes.bfloat16

S, B, D, T, E, H = 8, 32, 5, 30, 768, 128
H4 = 4 * H
N = B * D            # 160 sequences per stock
TOK = T * N          # 4800 tokens, t-major: tok = t*N + n
EB = E // 128        # 6 e-blocks
TPC = 3              # t-steps per xU chunk
CH = TPC * N         # 480 tokens per chunk
NCH = T // TPC       # 10 chunks
NCORES = 8
import os
USE_GP_CADJ = os.environ.get("USE_GP_CADJ", "1") == "1"
USE_GP_ATTN = os.environ.get("USE_GP_ATTN", "1") == "1"


# packed weight layout: (name, rows, cols) concatenated along the free dim
W_BF = [("wall", 128, H4), ("uall", 128, EB * H4), ("ident", 128, 128),
        ("a1w1", 128, H), ("a1w2", 128, H), ("a1vr", 128, 128),
        ("l2wih", 128, H4), ("l2whh", 128, H4),
        ("a2w1", 128, H), ("a2w2", 128, H), ("a2vr", 128, 128)]
W_F32 = [("wd", 128, H), ("bd", 128, 1), ("bg", 128, 4),
         ("a1b1", 128, 1), ("a1b2", 128, 1), ("a1b12", 128, 1), ("bl2", 128, 4),
         ("a2b1", 128, 1), ("a2b2", 128, 1),
         ("x1w", 128, H), ("x1b", 128, 1), ("x2w", 128, 64), ("x2b", 64, 1),
         ("sft", 17, B), ("h1w", 17, 64), ("h1b", 64, 1),
         ("h2w", 64, 32), ("h2b", 32, 1), ("hcw0", 32, S),
         ("hcb", 8, 1), ("hcws", 128, 4 * S)]
WBF_COLS = sum(c for _, _, c in W_BF)
WF32_COLS = sum(c for _, _, c in W_F32)

# gate permutation for the TimeLSTM: reference order (f, i, o, ct) -> (f, i, ct, o)
PERM1 = [0, 1, 3, 2]
# gate permutation for the day LSTM: reference order (i, f, g, o) -> (i, f, o, g)
PERM2 = [0, 1, 3, 2]


def _rep_ap(tile_ap, reps, inner):
    """AP reading [P, inner] tile as [P, reps, inner] with step-0 repeat."""
    return bass.AP(
        tensor=tile_ap.tensor,
        offset=tile_ap.offset,
        ap=[list(tile_ap.ap[0])] + [[0, reps], [1, inner]],
    )


def build_nc():
    nc = bacc.Bacc()

    def inp(name, shape, dtype=F32):
        return nc.declare_dram_parameter(name, shape, dtype, isOutput=False)

    x_h = inp("x", [EB, 128, TOK], BF16)
    tm1_h = inp("tm1", [1, TOK])
    mask_h = inp("maskbc", [1, TOK], BF16)
    m1_h = inp("m1", [1, TOK], BF16)
    wbf_h = inp("wbf", [128, WBF_COLS], BF16)
    wf32_h = inp("wf32", [128, WF32_COLS])

    out_h = nc.declare_dram_parameter("out", [S, B], F32, isOutput=True)

    # internal DRAM
    cc_in = nc.dram_tensor("cc_in", [64, B], F32)
    cc_out = nc.dram_tensor("cc_out", [S * 64, B], F32, addr_space="Shared")

    with TileContext(nc) as tc:
        with (
            tc.tile_pool(name="big", bufs=1) as big,
            tc.tile_pool(name="wpool", bufs=1) as wp,
            tc.tile_pool(name="state", bufs=1) as st,
            tc.tile_pool(name="xin", bufs=2) as xin,
            tc.tile_pool(name="work", bufs=2) as wk,
            tc.tile_pool(name="ps", bufs=2, space="PSUM") as ps,
        ):
            # ---------------- phase 0: weights (2 packed DMAs) ----------
            wbf_t = wp.tile([128, WBF_COLS], BF16, tag="wbf")
            nc.sync.dma_start(out=wbf_t[:, :], in_=wbf_h[:, :])
            wf32_t = wp.tile([128, WF32_COLS], F32, tag="wf32")
            nc.sync.dma_start(out=wf32_t[:, :], in_=wf32_h[:, :])

            def _mk_slices(table, tile):
                out, off = {}, 0
                for nm, rows, cols in table:
                    out[nm] = tile[0:rows, off : off + cols]
                    off += cols
                return out

            wsl = _mk_slices(W_BF, wbf_t)
            wsl.update(_mk_slices(W_F32, wf32_t))
            wall, uall, ident = wsl["wall"], wsl["uall"], wsl["ident"]
            a1w1, a1w2, a1vr = wsl["a1w1"], wsl["a1w2"], wsl["a1vr"]
            l2wih, l2whh = wsl["l2wih"], wsl["l2whh"]
            a2w1, a2w2, a2vr = wsl["a2w1"], wsl["a2w2"], wsl["a2vr"]
            wd, bd, bg, bl2 = wsl["wd"], wsl["bd"], wsl["bg"], wsl["bl2"]
            a1b1, a1b2, a2b1, a2b2 = wsl["a1b1"], wsl["a1b2"], wsl["a2b1"], wsl["a2b2"]
            a1b12 = wsl["a1b12"]
            x1w, x1b, x2w, x2b = wsl["x1w"], wsl["x1b"], wsl["x2w"], wsl["x2b"]
            sft, h1w, h1b = wsl["sft"], wsl["h1w"], wsl["h1b"]
            h2w, h2b, hcw0, hcb, hcws = wsl["h2w"], wsl["h2b"], wsl["hcw0"], wsl["hcb"], wsl["hcws"]

            maskbc = big.tile([128, TOK], BF16, tag="maskbc")
            tm1bc = big.tile([128, TOK], F32, tag="tm1bc")
            m1bc = big.tile([128, TOK], BF16, tag="m1bc")

            def tm1_load(ci):
                r = slice(ci * CH, (ci + 1) * CH)
                nc.scalar.dma_start(
                    out=tm1bc[:, r], in_=tm1_h[0:1, r].partition_broadcast(128)
                )

            # big persistent buffers
            xu = big.tile([128, 4 * TOK], BF16, tag="xu")
            obuf = big.tile([128, TOK], BF16, tag="obuf")

            # scan state
            h_bf = st.tile([128, N], BF16, tag="h_bf")
            c = st.tile([128, N], F32, tag="c")
            nc.vector.memzero(h_bf[:, :])
            nc.vector.memzero(c[:, :])

            # ------------- phases 1+2: xU production + scan -------------
            def xu_load(ci):
                t0 = ci * TPC
                # one consolidated chunk load: xT chunk [128, EB*CH]
                xT = xin.tile([128, EB * CH], BF16, tag="xTc")
                nc.sync.dma_start(
                    out=xT[:, :].rearrange("p (k c) -> p k c", k=EB),
                    in_=x_h[:, :, :].rearrange("k p c -> p k c")[
                        :, :, t0 * N : t0 * N + CH
                    ],
                )
                return xT

            def xu_j(ci, xT, j):
                # xu[j][chunk] = sum_k uall[k,j].T @ xT[k][chunk], bias folded
                t0 = ci * TPC
                pt = ps.tile([128, CH], F32, tag="xu")
                for k in range(EB):
                    nc.tensor.matmul(
                        pt[:, :],
                        uall[:, k * H4 + j * 128 : k * H4 + (j + 1) * 128],
                        xT[:, k * CH : k * CH + CH],
                        start=(k == 0),
                        stop=(k == EB - 1),
                    )
                dst = xu[:, j * TOK + t0 * N : j * TOK + t0 * N + CH]
                if j == 2:
                    nc.scalar.add(dst, pt[:, :], bg[:, j : j + 1])
                else:
                    nc.vector.tensor_scalar_add(dst, pt[:, :], bg[:, j : j + 1])

            def xu_chunk(ci):
                xT = xu_load(ci)
                for j in range(4):
                    xu_j(ci, xT, j)

            def scan_step(t):
                sl = slice(t * N, (t + 1) * N)
                # --- c-path first: depends only on c(t-1), overlaps the
                # previous step's h-path tail ---
                gB = ps.tile([128, 2 * N], F32, tag="gB")
                nc.tensor.matmul(
                    gB[:, N : 2 * N], wd[:, :], c[:, :],
                    start=True, stop=True, skip_group_check=True,
                )
                cs1 = st.tile([128, N], F32, tag="cs1")
                nc.scalar.activation(cs1[:, :], gB[:, N : 2 * N], AF.Tanh, bias=bd[:, 0:1])
                # c_adj = c + cs1 * tm1   (gpsimd, off the critical path)
                ca = st.tile([128, N], F32, tag="ca")
                nc.gpsimd.tensor_mul(ca[:, :], cs1[:, :], tm1bc[:, sl])
                nc.gpsimd.tensor_add(ca[:, :], ca[:, :], c[:, :])
                # --- h-path: gate matmuls need h_bf(t-1) ---
                gA = ps.tile([128, 3 * N], F32, tag="gA")
                for j in range(3):  # f, i, ct
                    r = slice(j * N, (j + 1) * N)
                    nc.tensor.matmul(
                        gA[:, r], wall[:, j * 128 : (j + 1) * 128], h_bf[:, :],
                        start=True, stop=False, skip_group_check=True,
                    )
                    nc.tensor.matmul(
                        gA[:, r], ident[:, :], xu[:, j * TOK + t * N : j * TOK + (t + 1) * N],
                        start=False, stop=True, skip_group_check=True,
                    )
                nc.tensor.matmul(
                    gB[:, 0:N], wall[:, 384:512], h_bf[:, :],
                    start=True, stop=False, skip_group_check=True,
                )
                nc.tensor.matmul(
                    gB[:, 0:N], ident[:, :], xu[:, 3 * TOK + t * N : 3 * TOK + (t + 1) * N],
                    start=False, stop=True, skip_group_check=True,
                )
                fic = st.tile([128, 3 * N], F32, tag="fic")
                nc.scalar.activation(fic[:, :], gA[:, :], AF.Sigmoid)
                nc.scalar.activation(obuf[:, sl], gB[:, 0:N], AF.Sigmoid)
                # c = f*c_adj + i*ct   (bv on gpsimd in parallel with av)
                av = st.tile([128, N], F32, tag="av")
                bv = st.tile([128, N], F32, tag="bv")
                nc.gpsimd.tensor_mul(bv[:, :], fic[:, N : 2 * N], fic[:, 2 * N : 3 * N])
                nc.vector.tensor_mul(av[:, :], fic[:, 0:N], ca[:, :])
                nc.vector.tensor_add(c[:, :], av[:, :], bv[:, :])
                tc2 = st.tile([128, N], BF16, tag="tc2")
                nc.scalar.activation(tc2[:, :], c[:, :], AF.Tanh)
                nc.vector.tensor_mul(h_bf[:, :], obuf[:, sl], tc2[:, :])

            # Fine-grained static interleave: the PE is in-order, so xU
            # matmuls for chunk ci+2 are woven between scan steps — while a
            # scan step waits on h_bf, the queued xU matmuls are NOT stuck
            # behind it across chunk boundaries (prefetch distance 2 keeps
            # every scan step's xu slice ready well in advance).
            nc.scalar.dma_start(
                out=m1bc[:, :], in_=m1_h[0:1, :].partition_broadcast(128)
            )
            hn = st.tile([128, N], F32, tag="hn")
            nc.vector.memzero(hn[:, :])

            def attn_weave(ci):
                # runs after chunk ci's scan steps: hn masked partial
                r = slice(ci * CH, (ci + 1) * CH)
                hm = wk.tile([128, CH], BF16, tag="hm")
                nc.vector.tensor_mul(hm[:, :], obuf[:, r], m1bc[:, r])
                hp = wk.tile([128, N], F32, tag="hp")
                nc.vector.tensor_reduce(
                    hp[:, :],
                    hm[:, :].rearrange("p (t n) -> p n t", t=TPC),
                    axis=mybir.AxisListType.X,
                    op=OP.add,
                )
                nc.vector.tensor_add(hn[:, :], hn[:, :], hp[:, :])

            tm1_load(0)
            tm1_load(1)
            xu_chunk(0)
            pre_xT = {1: xu_load(1)}
            xu_j(1, pre_xT[1], 0)
            xu_j(1, pre_xT[1], 1)
            for ci in range(NCH):
                for dt_ in range(TPC):
                    scan_step(ci * TPC + dt_)
                    if ci == 0 and dt_ < 2:
                        xu_j(1, pre_xT[1], 2 + dt_)
                    nxt = ci + 2
                    if nxt < NCH:
                        if dt_ == 0:
                            tm1_load(nxt)
                            pre_xT[nxt] = xu_load(nxt)
                            xu_j(nxt, pre_xT[nxt], 0)
                        elif dt_ == 1:
                            xu_j(nxt, pre_xT[nxt], 1)
                            xu_j(nxt, pre_xT[nxt], 2)
                        else:
                            xu_j(nxt, pre_xT.pop(nxt), 3)
                    if dt_ == TPC - 1:
                        attn_weave(ci)

            if os.environ.get("SKIP_TAIL", "0") == "1":
                osb0 = st.tile([S, B], F32, tag="osb0")
                nc.vector.tensor_copy(osb0[:, :], obuf[0:S, 0:B])
                nc.sync.dma_start(out=out_h[:, :], in_=osb0[:, :])
                return nc
            # attention-phase buffers (allocated after xT/xin released)
            nc.sync.dma_start(
                out=maskbc[:, :], in_=mask_h[0:1, :].partition_broadcast(128)
            )
            tmpbig = big.tile([128, TOK], BF16, tag="tmpbig")
            thout = big.tile([128, TOK], BF16, tag="thout")

            # ---------------- phase 3: attention over T -----------------
            hn_bf = st.tile([128, N], BF16, tag="hn_bf")
            nc.vector.tensor_copy(hn_bf[:, :], hn[:, :])
            # th = tanh(W2.T @ obuf + W1.T @ hn (repeated) + b1 + b2) per chunk;
            # the s1 broadcast rides the PE via a step-0-repeat rhs AP.
            for ci in range(NCH):
                r = slice(ci * CH, (ci + 1) * CH)
                sp = ps.tile([128, CH], F32, tag="mm")
                nc.tensor.matmul(sp[:, :], a1w2[:, :], obuf[:, r], start=True, stop=False, skip_group_check=True)
                nc.tensor.matmul(
                    sp[:, :].rearrange("p (r n) -> p r n", r=TPC),
                    a1w1[:, :], _rep_ap(hn_bf[:, :], TPC, N),
                    start=False, stop=True, skip_group_check=True,
                )
                nc.scalar.activation(thout[:, r], sp[:, :], AF.Tanh, bias=a1b12[:, 0:1])
            # scores replicated across partitions: lhsT = V tiled into all 128
            # columns, so out[p, tok] = sum_j V[j] th[j, tok] for every p.
            # The whole softmax then stays lane-local (no partition broadcast).
            neg30 = st.tile([128, 1], F32, tag="neg30")
            nc.vector.memset(neg30[:, :], -30.0)
            ewbc = big.tile([128, TOK], BF16, tag="m1bc")
            for ci in range(NCH):
                r = slice(ci * CH, (ci + 1) * CH)
                scp = ps.tile([128, CH], F32, tag="mm")
                nc.tensor.matmul(scp[:, :], a1vr[:, :], thout[:, r], start=True, stop=True)
                smc = wk.tile([128, CH], F32, tag="ti")
                nc.vector.scalar_tensor_tensor(
                    smc[:, :], scp[:, :], 30.0, maskbc[:, r], OP.add, OP.mult
                )
                nc.scalar.activation(ewbc[:, r], smc[:, :], AF.Exp, bias=neg30[:, 0:1])
            zr = st.tile([128, N], F32, tag="zr")
            nc.vector.tensor_reduce(
                zr[:, :],
                ewbc[:, :].rearrange("p (t n) -> p n t", t=T),
                axis=mybir.AxisListType.X,
                op=OP.add,
            )
            rz = st.tile([128, N], F32, tag="rz")
            nc.vector.reciprocal(rz[:, :], zr[:, :])
            nc.vector.tensor_mul(tmpbig[:, :], obuf[:, :], ewbc[:, :])
            ctxr = st.tile([128, N], F32, tag="ctxr")
            nc.vector.tensor_reduce(
                ctxr[:, :],
                tmpbig[:, :].rearrange("p (t n) -> p n t", t=T),
                axis=mybir.AxisListType.X,
                op=OP.add,
            )
            ctx_bf = st.tile([128, N], BF16, tag="ctx_bf")
            nc.vector.tensor_mul(ctx_bf[:, :], ctxr[:, :], rz[:, :])

            # ---------------- phase 4: day LSTM (D steps) ---------------
            hs_bf = st.tile([128, N], BF16, tag="hs_bf")
            h2st = st.tile([128, B], BF16, tag="h2st")
            c2st = st.tile([128, B], F32, tag="c2st")
            nc.vector.memzero(h2st[:, :])
            nc.vector.memzero(c2st[:, :])
            for d in range(D):
                xin_d = ctx_bf[:, :].rearrange("p (b d) -> p d b", d=D)[:, d, :]
                g2 = ps.tile([128, 4 * B], F32, tag="mm")
                for j in range(4):
                    r = slice(j * B, (j + 1) * B)
                    nc.tensor.matmul(
                        g2[:, r], l2wih[:, j * 128 : (j + 1) * 128], xin_d,
                        start=True, stop=False, skip_group_check=True,
                    )
                    nc.tensor.matmul(
                        g2[:, r], l2whh[:, j * 128 : (j + 1) * 128], h2st[:, :],
                        start=False, stop=True, skip_group_check=True,
                    )
                sg = st.tile([128, 3 * B], F32, tag="sg")
                for j in range(3):  # i, f, o
                    nc.scalar.activation(
                        sg[:, j * B : (j + 1) * B], g2[:, j * B : (j + 1) * B],
                        AF.Sigmoid, bias=bl2[:, j : j + 1],
                    )
                tg = st.tile([128, B], F32, tag="tg")
                nc.scalar.activation(tg[:, :], g2[:, 3 * B : 4 * B], AF.Tanh, bias=bl2[:, 3:4])
                a2v_ = st.tile([128, B], F32, tag="a2v_")
                b2v_ = st.tile([128, B], F32, tag="b2v_")
                nc.vector.tensor_mul(a2v_[:, :], sg[:, B : 2 * B], c2st[:, :])
                nc.vector.tensor_mul(b2v_[:, :], sg[:, 0:B], tg[:, :])
                nc.vector.tensor_add(c2st[:, :], a2v_[:, :], b2v_[:, :])
                tc2b = st.tile([128, B], BF16, tag="tc2b")
                nc.scalar.activation(tc2b[:, :], c2st[:, :], AF.Tanh)
                nc.vector.tensor_mul(h2st[:, :], sg[:, 2 * B : 3 * B], tc2b[:, :])
                nc.vector.tensor_copy(
                    hs_bf[:, :].rearrange("p (b d) -> p d b", d=D)[:, d, :], h2st[:, :]
                )

            # ---------------- phase 5: attention over D -----------------
            s1aps = ps.tile([128, B], F32, tag="mm")
            nc.tensor.matmul(s1aps[:, :], a2w1[:, :], h2st[:, :], start=True, stop=True)
            s1a = st.tile([128, B], F32, tag="s1a")
            nc.scalar.add(s1a[:, :], s1aps[:, :], a2b1[:, 0:1])
            s2aps = ps.tile([128, N], F32, tag="mm")
            nc.tensor.matmul(s2aps[:, :], a2w2[:, :], hs_bf[:, :], start=True, stop=True)
            t2i = st.tile([128, N], F32, tag="t2i")
            # hs layout is (b, d): s1a must repeat per-b along d -> use [b][d] view
            nc.vector.tensor_add(
                t2i[:, :].rearrange("p (b d) -> p b d", d=D),
                s2aps[:, :].rearrange("p (b d) -> p b d", d=D),
                bass.AP(
                    tensor=s1a.tensor,
                    offset=s1a[:, :].offset,
                    ap=[list(s1a[:, :].ap[0])] + [[1, B], [0, D]],
                ),
            )
            th2 = st.tile([128, N], BF16, tag="th2")
            nc.scalar.activation(th2[:, :], t2i[:, :], AF.Tanh, bias=a2b2[:, 0:1])
            # replicated scores again: out[p, (b,d)] = sum_j V2[j] th2[j, (b,d)]
            sc2p = ps.tile([128, N], F32, tag="mm")
            nc.tensor.matmul(sc2p[:, :], a2vr[:, :], th2[:, :], start=True, stop=True)
            ew2r = st.tile([128, N], BF16, tag="ew2r")
            nc.scalar.activation(ew2r[:, :], sc2p[:, :], AF.Exp)
            z2r = st.tile([128, B], F32, tag="z2r")
            nc.vector.tensor_reduce(
                z2r[:, :],
                ew2r[:, :].rearrange("p (b d) -> p b d", d=D),
                axis=mybir.AxisListType.X,
                op=OP.add,
            )
            rz2 = st.tile([128, B], F32, tag="rz2")
            nc.vector.reciprocal(rz2[:, :], z2r[:, :])
            tmp2 = st.tile([128, N], BF16, tag="tmp2")
            nc.vector.tensor_mul(tmp2[:, :], hs_bf[:, :], ew2r[:, :])
            ctx2r = st.tile([128, B], F32, tag="ctx2r")
            nc.vector.tensor_reduce(
                ctx2r[:, :],
                tmp2[:, :].rearrange("p (b d) -> p b d", d=D),
                axis=mybir.AxisListType.X,
                op=OP.add,
            )
            ctx2 = st.tile([128, B], F32, tag="ctx2")
            nc.vector.tensor_mul(ctx2[:, :], ctx2r[:, :], rz2[:, :])

            # ---------------- phase 6: per-stock head + global ----------
            y1ps = ps.tile([128, B], F32, tag="mm")
            nc.tensor.matmul(y1ps[:, :], x1w[:, :], ctx2[:, :], start=True, stop=True)
            y1 = st.tile([128, B], F32, tag="y1")
            nc.scalar.activation(y1[:, :], y1ps[:, :], AF.Relu, bias=x1b[:, 0:1])
            o2ps = ps.tile([64, B], F32, tag="mm")
            nc.tensor.matmul(o2ps[:, :], x2w[:, :], y1[:, :], start=True, stop=True)
            txt = st.tile([64, B], F32, tag="txt")
            nc.scalar.add(txt[:, :], o2ps[:, :], x2b[:, 0:1])
            nc.sync.dma_start(out=cc_in[:, :], in_=txt[:, :])
            nc.gpsimd.collective_compute(
                "AllGather",
                OP.bypass,
                replica_groups=[list(range(NCORES))],
                ins=[cc_in[:, :]],
                outs=[cc_out[:, :]],
            )
            # xs path
            y2ps = ps.tile([64, B], F32, tag="mm")
            nc.tensor.matmul(y2ps[:, :], h1w[:, :], sft[:, :], start=True, stop=True)
            y2 = st.tile([64, B], F32, tag="y2")
            nc.scalar.activation(y2[:, :], y2ps[:, :], AF.Relu, bias=h1b[:, 0:1])
            xsps = ps.tile([32, B], F32, tag="mm")
            nc.tensor.matmul(xsps[:, :], h2w[:, :], y2[:, :], start=True, stop=True)
            xst = st.tile([32, B], F32, tag="xst")
            nc.scalar.add(xst[:, :], xsps[:, :], h2b[:, 0:1])
            # final: out.T = tanh(hc_W.T @ [xs; text].T + hc_b)
            ga = st.tile([128, 4 * B], F32, tag="ga")
            nc.sync.dma_start(
                out=ga[:, :].rearrange("p (k b) -> p k b", k=4),
                in_=cc_out[:, :].rearrange("(k p) b -> p k b", k=4),
            )
            fps = ps.tile([S, B], F32, tag="mm")
            nc.tensor.matmul(fps[:, :], hcw0[:, :], xst[:, :], start=True, stop=False)
            for k in range(4):
                nc.tensor.matmul(
                    fps[:, :], hcws[:, k * S : (k + 1) * S], ga[:, k * B : (k + 1) * B],
                    start=False, stop=(k == 3),
                )
            osb = st.tile([S, B], F32, tag="osb")
            nc.scalar.activation(osb[:, :], fps[:, :], AF.Tanh, bias=hcb[:, 0:1])
            nc.sync.dma_start(out=out_h[:, :], in_=osb[:, :])

    return nc


def make_in_maps(
    stock_feats, sentence_feat, time_feats, len_tweets,
    tl_Wall, tl_ball, tl_Uall, tl_bU, tl_Wd, tl_bd,
    a1_W1, a1_b1, a1_W2, a1_b2, a1_V, a1_bV,
    l2_Wih, l2_bih, l2_Whh, l2_bhh,
    a2_W1, a2_b1, a2_W2, a2_b2, a2_V, a2_bV,
    x1_W, x1_b, x2_W, x2_b,
    h1_W, h1_b, h2_W, h2_b, hc_W, hc_b,
):
    f32 = np.float32

    def permcols(w, perm):
        # w [..., 4*128] -> permuted gate blocks
        shp = w.shape
        wr = w.reshape(shp[:-1] + (4, 128))
        return wr[..., perm, :].reshape(shp)

    in_maps = []
    shared = {}
    shared["sft"] = np.ascontiguousarray(stock_feats.T).astype(f32)
    shared["h1w"] = np.asarray(h1_W, f32)
    shared["h1b"] = np.asarray(h1_b, f32).reshape(64, 1)
    shared["h2w"] = np.asarray(h2_W, f32)
    shared["h2b"] = np.asarray(h2_b, f32).reshape(32, 1)
    shared["hcw0"] = np.asarray(hc_W, f32)[:32]
    shared["hcws"] = np.ascontiguousarray(
        np.asarray(hc_W, f32)[32:].reshape(4, 128, S).transpose(1, 0, 2)
    ).reshape(128, 4 * S)
    shared["hcb"] = np.asarray(hc_b, f32).reshape(S, 1)
    shared["ident"] = np.eye(128, dtype=f32).astype(BF)

    for s in range(S):
        m = dict(shared)
        xs = np.asarray(sentence_feat[:, s], f32)          # [B, D, T, E]
        xbf = xs.astype(BF)                                # cast first (cheap)
        # [B, D, T, E] -> [E, T, B, D] -> [EB, 128, T*N]
        m["x"] = np.ascontiguousarray(xbf.transpose(3, 2, 0, 1)).reshape(EB, 128, TOK)
        tt = np.asarray(time_feats[:, s], f32)             # [B, D, T]
        m["tm1"] = (
            np.ascontiguousarray(tt.transpose(2, 0, 1)).reshape(1, TOK) - 1.0
        ).astype(f32)
        lens = np.asarray(len_tweets[:, s]).reshape(N)     # [N] int
        tgrid = np.arange(T)[:, None]
        m["maskbc"] = (tgrid < lens[None, :]).astype(f32).reshape(1, TOK).astype(BF)
        m["m1"] = (tgrid == (lens[None, :] - 1)).astype(f32).reshape(1, TOK).astype(BF)
        m["wd"] = np.asarray(tl_Wd[s], f32)
        m["bd"] = np.asarray(tl_bd[s], f32).reshape(H, 1)
        m["wall"] = permcols(np.asarray(tl_Wall[s], f32), PERM1).astype(BF)
        u = permcols(np.asarray(tl_Uall[s], f32), PERM1)   # [E, 512]
        m["uall"] = np.ascontiguousarray(
            u.reshape(EB, 128, H4).transpose(1, 0, 2)
        ).reshape(128, EB * H4).astype(BF)
        bgv = permcols(
            (np.asarray(tl_ball[s], f32) + np.asarray(tl_bU[s], f32))[None, :], PERM1
        )[0]
        m["bg"] = np.ascontiguousarray(bgv.reshape(4, 128).T).astype(f32)
        m["a1w1"] = np.asarray(a1_W1[s], f32).astype(BF)
        m["a1b1"] = np.asarray(a1_b1[s], f32).reshape(H, 1)
        m["a1w2"] = np.asarray(a1_W2[s], f32).astype(BF)
        m["a1b2"] = np.asarray(a1_b2[s], f32).reshape(H, 1)
        m["a1b12"] = (np.asarray(a1_b1[s], f32) + np.asarray(a1_b2[s], f32)).reshape(H, 1)
        m["a1vr"] = np.tile(np.asarray(a1_V[s], f32).reshape(H, 1), (1, 128)).astype(BF)
        m["l2wih"] = permcols(np.asarray(l2_Wih[s], f32), PERM2).astype(BF)
        m["l2whh"] = permcols(np.asarray(l2_Whh[s], f32), PERM2).astype(BF)
        bl2v = permcols(
            (np.asarray(l2_bih[s], f32) + np.asarray(l2_bhh[s], f32))[None, :], PERM2
        )[0]
        m["bl2"] = np.ascontiguousarray(bl2v.reshape(4, 128).T).astype(f32)
        m["a2w1"] = np.asarray(a2_W1[s], f32).astype(BF)
        m["a2b1"] = np.asarray(a2_b1[s], f32).reshape(H, 1)
        m["a2w2"] = np.asarray(a2_W2[s], f32).astype(BF)
        m["a2b2"] = np.asarray(a2_b2[s], f32).reshape(H, 1)
        m["a2vr"] = np.tile(np.asarray(a2_V[s], f32).reshape(H, 1), (1, 128)).astype(BF)
        m["x1w"] = np.asarray(x1_W[s], f32)
        m["x1b"] = np.asarray(x1_b[s], f32).reshape(H, 1)
        m["x2w"] = np.asarray(x2_W[s], f32)
        m["x2b"] = np.asarray(x2_b[s], f32).reshape(64, 1)
        wbf = np.zeros((128, WBF_COLS), BF)
        off = 0
        for nm, rows, cols in W_BF:
            v = np.asarray(m.pop(nm))
            wbf[:rows, off : off + cols] = v
            off += cols
        m["wbf"] = wbf
        wf32 = np.zeros((128, WF32_COLS), f32)
        off = 0
        for nm, rows, cols in W_F32:
            v = np.asarray(m.pop(nm), f32).reshape(rows, cols)
            wf32[:rows, off : off + cols] = v
            off += cols
        m["wf32"] = wf32
        in_maps.append(m)
    return in_maps


_CACHED_NC = None
TRACE = False
LAST_EXEC_NS = None
LAST_RESULT = None


def kernel(**inputs) -> np.ndarray:
    global _CACHED_NC, LAST_EXEC_NS, LAST_RESULT
    from concourse.bass_utils import run_bass_kernel_spmd

    in_maps = make_in_maps(**inputs)
    if _CACHED_NC is None:
        nc = build_nc()
        nc.finalize()
        _CACHED_NC = nc
    res = run_bass_kernel_spmd(
        _CACHED_NC, in_maps, list(range(NCORES)), trace=TRACE
    )
    LAST_EXEC_NS = res.exec_time_ns
    LAST_RESULT = res
    out_t = res.results[0]["out"]          # [S, B]
    return np.ascontiguousarray(out_t.T).astype(np.float32)  # [B, S]
